# revision 72
# baseline (speedup 1.0000x reference)
"""Trainium2 Bass kernel for a dense transformer block (RMSNorm -> causal MHA
-> residual -> RMSNorm -> SwiGLU FFN -> residual).

Sharding: data-parallel over tokens with a *strided* assignment -- core c
owns every token position == c (mod 8) of both batches (512 tokens/core),
weights replicated. The stride makes the causal chunk structure identical on
every core (no padding waste), so one SPMD program serves all cores; the
residual per-core causality lands in small per-core mask *data* for the
diagonal key chunks only. K/V are exchanged with one AllGather pair.

Matmuls run in bf16 with fp32 PSUM accumulation; residuals/norms stay fp32.
Attention processes chunk pairs for both batches at once ([128, 512] exp and
mask tiles), softmax denominators accumulate on the PE via a ones-column
matmul, and 1/r normalization is applied via a PE partition-broadcast.
"""

from dataclasses import dataclass
from contextlib import ExitStack

import numpy as np

import concourse.bacc as bacc
import concourse.mybir as mybir
import concourse.tile as tile
from concourse.bass_utils import run_bass_kernel_spmd

try:
    import ml_dtypes

    BF16 = ml_dtypes.bfloat16
    E4M3 = ml_dtypes.float8_e4m3  # TRN fp8e4: max +-240, matches <=240
except ImportError:  # pragma: no cover
    import jax.numpy as jnp

    BF16 = jnp.bfloat16
    E4M3 = jnp.float8_e4m3

F32 = mybir.dt.float32
BF = mybir.dt.bfloat16
FP8 = mybir.dt.float8e4
AF = mybir.ActivationFunctionType
DR = mybir.MatmulPerfMode.DoubleRow

# fp8 scaling: weights/activations are pre-scaled out of the subnormal range
# (w sigma ~ 1/sqrt(D), xn ~ unit RMS); descale on PSUM copy-out.
S_W = 128.0  # qkv / out weight scale
S_X = 16.0   # normalized-activation scale
S_O = 16.0   # attention-output scale
S_V = 16.0   # V scale for fp8 attention weights
S_QK = 16.0  # Q/K scale for fp8 scores
C_SHIFT = 3.0  # exp shift: e' = exp(s/sqrt(Hd) - C); cancels in softmax


@dataclass(frozen=True)
class Cfg:
    B: int = 2
    S: int = 2048
    D: int = 2048
    H: int = 16
    DFF: int = 8192
    n_cores: int = 8
    eps: float = 1e-6
    use_silu: bool = False  # ACT Silu table vs sigmoid+mul (sim lacks Silu)
    fake_ag: bool = False   # replace AllGather with local DMA (TimelineSim)
    fp8_qkv: bool = True    # qkv projection in fp8e4 DoubleRow (2x PE)
    fp8_out: bool = True    # attention out-projection in fp8e4 DoubleRow
    pool_rsum: bool = False  # softmax denominators: accumulate e on Pool
    fp8_att: bool = True    # shifted-exp fp8 attn weights + fp8 V (DR av/rsum)
    fp8_qk: bool = True     # fp8 Q/K (fp8 scores; K travels fp8 on the wire)
    merged_ag: bool = False  # single AllGather carrying K and V (slower:
    #   2MB/rank leaves the <1MB low-latency mesh collective regime)
    hh_sbuf: bool = True    # keep FFN hidden activations SBUF-resident
    stop_after: int = 0     # timing probe: truncate after phase N (0 = full)

    @property
    def Hd(self):
        return self.D // self.H

    @property
    def S_blk(self):
        return self.S // self.n_cores  # per-(core,batch) query block

    @property
    def T(self):
        return self.B * self.S_blk  # tokens per core

    @property
    def TB(self):
        return self.T // 128

    @property
    def DC(self):
        return self.D // 128

    @property
    def NK(self):
        return self.S // 128  # padded key chunks per batch

    @property
    def QB(self):
        return self.S_blk

    @property
    def FFB(self):
        return self.DFF // 128

    @property
    def CT(self):
        return max(1, self.D // 512)

    @property
    def CW(self):
        return min(512, self.D)

    @property
    def GW(self):
        return min(512, self.D)  # qkv weight col group width

    @property
    def W2G(self):
        return 4  # w2 chunks per DMA group


FULL = Cfg()


def build_nc(cfg: Cfg):
    """Build the per-core Bass program (identical on all cores)."""
    assert cfg.Hd == 128 and cfg.S_blk % 128 == 0 and cfg.T % 128 == 0
    n, D, H, T, DC, NK, QB, FFB = (
        cfg.n_cores, cfg.D, cfg.H, cfg.T, cfg.DC, cfg.NK, cfg.QB, cfg.FFB)
    S_blk, TB, CT, CW, DFF, GW, W2G = (
        cfg.S_blk, cfg.TB, cfg.CT, cfg.CW, cfg.DFF, cfg.GW, cfg.W2G)
    SB2 = S_blk // 128
    assert FFB % W2G == 0 and (3 * D) % GW == 0
    assert n % 4 == 0 and T % 256 == 0

    nc = bacc.Bacc("TRN2", target_bir_lowering=False, debug=False,
                   num_devices=n)

    # ---- I/O (host pre-tiled layouts; one DMA per weight group) ----
    x_io = nc.dram_tensor("x_c", [T, D], F32, kind="ExternalInput")
    wqkv_io = nc.dram_tensor("wqkv_t", [3 * D // GW, 128, DC, GW],
                             FP8 if cfg.fp8_qkv else BF,
                             kind="ExternalInput")
    wout_io = nc.dram_tensor("wout_t", [CT, 128, DC, CW],
                             FP8 if cfg.fp8_out else BF,
                             kind="ExternalInput")
    w13_io = nc.dram_tensor("w13_t", [FFB // 2, 128, DC, 512], BF,
                            kind="ExternalInput")
    w2_io = nc.dram_tensor("w2_t", [FFB // W2G, 128, W2G, D], BF,
                           kind="ExternalInput")
    masks_io = nc.dram_tensor("masks", [128, n // 2, 2, 256],
                              FP8 if cfg.fp8_att else BF,
                              kind="ExternalInput")
    ident_io = nc.dram_tensor("ident", [128, 128], BF, kind="ExternalInput")
    ones_io = nc.dram_tensor("ones128", [128, 128], BF, kind="ExternalInput")
    out_io = nc.dram_tensor("out_c", [T, D], F32, kind="ExternalOutput")

    SA = cfg.stop_after if cfg.stop_after else 99  # phase truncation probe
    with tile.TileContext(nc) as tc, ExitStack() as top:
        P = top.enter_context(tc.tile_pool(name="persist", bufs=1))
        consts = top.enter_context(tc.tile_pool(name="consts", bufs=1))

        ident = consts.tile([128, 128], BF, name="ident_sb")
        ones = consts.tile([128, 128], BF, name="ones_sb")
        mask_sb = consts.tile([128, n // 2, 2, 256],
                              FP8 if cfg.fp8_att else BF, name="mask_sb")
        eps_t = consts.tile([128, 1], F32, name="eps_sb")
        nc.sync.dma_start(ident[:], ident_io[:, :])
        nc.any.memset(eps_t[:], cfg.eps)
        if cfg.fp8_att:
            # all-ones fp8 weights, 128 out-columns: the denominator matmul
            # rp = ones8^T @ e lands broadcast across all 128 partitions
            ones8 = consts.tile([128, 2, 128], FP8, name="ones8_sb")
            nc.any.memset(ones8[:], 1.0)
            negc = consts.tile([128, 1], F32, name="negc_sb")
            nc.any.memset(negc[:], -C_SHIFT)

        x1_t = [P.tile([128, D], F32, name=f"x1_{tb}") for tb in range(TB)]
        xzt = P.tile([128, DC, T], BF, name="xzt")       # z2^T for FFN
        xzt8 = (P.tile([128, DC, T], FP8, name="xzt8")   # xn^T fp8 for QKV
                if cfg.fp8_qkv else xzt)

        # attention-scoped persistents: freed before the FFN phases so the
        # SwiGLU hidden state can live in SBUF
        attn_stack = ExitStack()
        AP = attn_stack.enter_context(tc.tile_pool(name="attnP", bufs=1))
        QKDT = FP8 if cfg.fp8_qk else BF
        qt = AP.tile([128, H, T], QKDT, name="qt")       # Q d-major
        v_all = AP.tile([128, TB, D], FP8 if cfg.fp8_att else BF,
                        name="v_all")                    # V token-major
        ot = AP.tile([128, H, T], FP8 if cfg.fp8_out else BF,
                     name="ot")                          # attn out d-major

        dram = top.enter_context(tc.tile_pool(name="dram", bufs=1, space="DRAM"))
        VDT = FP8 if cfg.fp8_att else BF
        if cfg.merged_ag:
            assert cfg.fp8_qk and cfg.fp8_att
            # fp8 [4096, 512]: rows [0, D) = K^T; rows [D, 2D) = V flattened
            # (token t = 4 rows of 512 features each)
            kv_c = dram.tile([2 * D, T], FP8, name="kv_contrib")
            kv_g = dram.tile([n * 2 * D, T], FP8, name="kv_gath",
                             addr_space="Shared")
        else:
            kt_c = dram.tile([D, T], QKDT, name="kt_contrib")
            v_c = dram.tile([T, D], VDT, name="v_contrib")
            kt_g = dram.tile([n * D, T], QKDT, name="kt_gath",
                             addr_space="Shared")
            v_g = dram.tile([n * T, D], VDT, name="v_gath",
                            addr_space="Shared")
        if not cfg.hh_sbuf:
            hh_d = dram.tile([DFF, T], BF, name="hh_d")  # swiglu spill

        def rmsnorm_transpose(get_src, ps_tp, pool, dst, out_scale=1.0):
            """token-major fp32 tiles -> d-major bf16/fp8 into dst."""
            for tb in range(TB):
                xt = get_src(tb)
                scr = pool.tile([128, D], BF, name="nrm_scr")
                ssq = pool.tile([128, 1], F32, name="nrm_ssq")
                nc.scalar.activation(scr[:], xt[:], AF.Square,
                                     accum_out=ssq[:])
                sd = pool.tile([128, 1], F32, name="nrm_sd")
                nc.scalar.activation(sd[:], ssq[:], AF.Sqrt,
                                     bias=eps_t[:], scale=1.0 / D)
                inv = pool.tile([128, 1], F32, name="nrm_inv")
                nc.vector.reciprocal(inv[:], sd[:])
                xn = pool.tile([128, D], BF, name="nrm_xn")
                nc.vector.tensor_scalar_mul(xn[:], xt[:], inv[:])
                for dc in range(DC):
                    tp = ps_tp.tile([128, 128], BF, name="tp")
                    nc.tensor.transpose(tp[:], xn[:, dc * 128:(dc + 1) * 128],
                                        ident[:])
                    d = dst[:, dc, tb * 128:(tb + 1) * 128]
                    if out_scale != 1.0:
                        nc.scalar.mul(d, tp[:], out_scale)
                    else:
                        nc.vector.tensor_copy(d, tp[:])

        # ================= phase 1: rmsnorm1 + transpose =================
        with tc.tile_pool(name="ph1", bufs=2) as ph1, \
             tc.tile_pool(name="ps_tp1", bufs=4, space="PSUM") as ps_tp1:

            def load_x(tb):
                t = ph1.tile([128, D], F32, name=f"xld_{tb}")
                nc.sync.dma_start(t[:], x_io[tb * 128:(tb + 1) * 128, :])
                return t

            rmsnorm_transpose(load_x, ps_tp1, ph1, xzt8,
                              S_X if cfg.fp8_qkv else 1.0)
            # consts not needed until attention -- keep them off the DMA
            # queue ahead of the x loads
            nc.sync.dma_start(ones[:], ones_io[:, :])
            nc.sync.dma_start(mask_sb[:], masks_io[:, :, :, :])

        # ============== phase 2: qkv projections + AllGather ==============
        # order: K, V (feed the AllGather), then Q (overlaps the AG)
        with tc.tile_pool(name="ph2", bufs=1) as ph2, \
             tc.tile_pool(name="wg", bufs=3) as wg, \
             tc.tile_pool(name="ps_mm", bufs=3, space="PSUM") as ps_mm:
            kt_l = ph2.tile([128, H, T], QKDT, name="kt_l")  # local K d-major
            qkscale = ((S_QK if cfg.fp8_qk else 1.0)
                       / ((S_W * S_X) if cfg.fp8_qkv else 1.0))

            def load_group(g):
                t = wg.tile([128, DC, GW], FP8 if cfg.fp8_qkv else BF,
                            name="wg_t")
                nc.sync.dma_start(t[:], wqkv_io[g, :, :, :])
                return t

            def qk_block(wt, off, dst, slot):
                ps = ps_mm.tile([128, T], F32, name="ps_qk")
                if cfg.fp8_qkv:
                    for dc in range(0, DC, 2):
                        nc.tensor.matmul(ps[:], wt[:, dc:dc + 2, off:off + 128],
                                         xzt8[:, dc:dc + 2, :],
                                         start=(dc == 0), stop=(dc == DC - 2),
                                         perf_mode=DR)
                else:
                    for dc in range(DC):
                        nc.tensor.matmul(ps[:], wt[:, dc, off:off + 128],
                                         xzt[:, dc, :],
                                         start=(dc == 0), stop=(dc == DC - 1))
                nc.scalar.mul(dst[:, slot, :], ps[:], qkscale)

            # K: wqkv cols [D, 2D)
            for g in range(D // GW, 2 * D // GW if SA >= 2 else 0):
                wt = load_group(g)
                for b in range(GW // 128):
                    h = g * GW // 128 + b - H
                    qk_block(wt, b * 128, kt_l, h)
            # V: wqkv cols [2D, 3D) -> token-major
            for g in range(2 * D // GW, 3 * D // GW if SA >= 2 else 0):
                wt = load_group(g)
                base = g * GW - 2 * D
                vscale = ((S_V if cfg.fp8_att else 1.0)
                          / ((S_W * S_X) if cfg.fp8_qkv else 1.0))
                for tb in range(TB):
                    for c0 in range(0, GW, CW):
                        ps = ps_mm.tile([128, CW], F32, name="ps_v")
                        if cfg.fp8_qkv:
                            for dc in range(0, DC, 2):
                                nc.tensor.matmul(
                                    ps[:],
                                    xzt8[:, dc:dc + 2, tb * 128:(tb + 1) * 128],
                                    wt[:, dc:dc + 2, c0:c0 + CW],
                                    start=(dc == 0), stop=(dc == DC - 2),
                                    perf_mode=DR)
                        else:
                            for dc in range(DC):
                                nc.tensor.matmul(
                                    ps[:], xzt[:, dc, tb * 128:(tb + 1) * 128],
                                    wt[:, dc, c0:c0 + CW],
                                    start=(dc == 0), stop=(dc == DC - 1))
                        nc.scalar.mul(
                            v_all[:, tb, base + c0:base + c0 + CW], ps[:],
                            vscale)

            for h in range(H if SA >= 2 else 0):
                dst = kv_c if cfg.merged_ag else kt_c
                nc.sync.dma_start(dst[h * 128:(h + 1) * 128, :],
                                  kt_l[:, h, :])
            for tb in range(TB if SA >= 2 else 0):
                if cfg.merged_ag:
                    nc.sync.dma_start(
                        kv_c[D + tb * 512:D + (tb + 1) * 512, :]
                        .rearrange("(p a) q -> p a q", a=4),
                        v_all[:, tb, :]
                        .rearrange("p (a q) -> p a q", a=4))
                else:
                    nc.sync.dma_start(v_c[tb * 128:(tb + 1) * 128, :],
                                      v_all[:, tb, :])
            if SA < 2:
                pass
            elif cfg.fake_ag:
                if cfg.merged_ag:
                    nc.sync.dma_start(kv_g[0:2 * D, :], kv_c[:, :])
                else:
                    nc.sync.dma_start(kt_g[0:D, :], kt_c[:, :])
                    nc.sync.dma_start(v_g[0:T, :], v_c[:, :])
            elif cfg.merged_ag:
                nc.gpsimd.collective_compute(
                    "AllGather", mybir.AluOpType.bypass,
                    replica_groups=[list(range(n))],
                    ins=[kv_c.opt()], outs=[kv_g.opt()])
            else:
                nc.gpsimd.collective_compute(
                    "AllGather", mybir.AluOpType.bypass,
                    replica_groups=[list(range(n))],
                    ins=[kt_c.opt()], outs=[kt_g.opt()])
                nc.gpsimd.collective_compute(
                    "AllGather", mybir.AluOpType.bypass,
                    replica_groups=[list(range(n))],
                    ins=[v_c.opt()], outs=[v_g.opt()])

            # Q: wqkv cols [0, D)  (overlaps the AG)
            for g in range(0, D // GW if SA >= 2 else 0):
                wt = load_group(g)
                for b in range(GW // 128):
                    h = g * GW // 128 + b
                    qk_block(wt, b * 128, qt, h)

        # ================= phase 4: attention =================
        # Strided token assignment: core c owns tokens == c (mod n); local
        # order is l = blk*256 + batch*128 + i  (token = n*(blk*128+i) + c).
        # Every local 256-query super-block b needs exactly (b+1)*n key
        # chunks on every core -- no padding. Only diagonal (j2 == b) chunks
        # need a mask; mask content is per-core data. Chunks are processed
        # in pairs so exp/mask run on [128, 512] tiles.
        HG = min(8, H)
        NP2 = n // 2
        SPG = 2 if cfg.fp8_att else 1  # score pairs fused per exp call
        with tc.tile_pool(name="kv", bufs=2) as kv, \
             tc.tile_pool(name="ktp", bufs=4) as ktp, \
             tc.tile_pool(name="esb", bufs=6 // SPG) as esb, \
             tc.tile_pool(name="esump", bufs=2) as esump, \
             tc.tile_pool(name="aux", bufs=2) as aux, \
             tc.tile_pool(name="ps_s", bufs=4 // SPG, space="PSUM") as ps_s, \
             tc.tile_pool(name="ps_ot", bufs=1, space="PSUM") as ps_ot, \
             tc.tile_pool(name="ps_r", bufs=1, space="PSUM") as ps_r, \
             tc.tile_pool(name="ps_b", bufs=1, space="PSUM") as ps_b:
            for hq in range(H // HG if SA >= 4 else 0):
                vtb = [kv.tile([128, NK, HG * 128],
                               FP8 if cfg.fp8_att else BF, name=f"vtb{qb}")
                       for qb in range(2)]
                for qb in range(2):
                    for r in range(n):
                        for j2 in range(SB2):
                            t0 = j2 * 256 + qb * 128
                            if cfg.merged_ag:
                                base = r * 2 * D + D
                                src = (kv_g[base + 4 * t0:
                                            base + 4 * (t0 + 128), :]
                                       .rearrange("(p a) q -> p a q", a=4)
                                       [:, hq, :])
                            else:
                                src = v_g[r * T + t0:r * T + t0 + 128,
                                          hq * HG * 128:(hq + 1) * HG * 128]
                            nc.sync.dma_start(vtb[qb][:, j2 * n + r, :], src)
                for hi in range(HG):
                    h = hq * HG + hi
                    ktb = ktp.tile([128, n, T], QKDT, name="ktb")
                    for r in range(n):
                        if cfg.merged_ag:
                            src = kv_g[r * 2 * D + h * 128:
                                       r * 2 * D + (h + 1) * 128, :]
                        else:
                            src = kt_g[r * D + h * 128:r * D + (h + 1) * 128,
                                       :]
                        nc.sync.dma_start(ktb[:, r, :], src)
                    for b in range(SB2):
                        otp = [ps_ot.tile([128, 128], F32, name=f"otp{qb}")
                               for qb in range(2)]
                        rp = ps_r.tile(
                            [128, 256] if cfg.fp8_att else [1, 512], F32,
                            name="rp")
                        npairs = (b + 1) * NP2
                        pi = 0
                        for j2 in range(b + 1 if cfg.fp8_att else 0):
                            # fp8 path: 2 pairs (4 key chunks) per exp/mask
                            for pg in range(NP2 // SPG):
                                sp = ps_s.tile([128, SPG, 2, 256], F32,
                                               name="sp")
                                for c in range(2 * SPG):
                                    r = 2 * SPG * pg + c
                                    for qb in range(2):
                                        nc.tensor.matmul(
                                            sp[:, c // 2, c % 2,
                                               qb * 128:(qb + 1) * 128],
                                            ktb[:, r,
                                                j2 * 256 + qb * 128:
                                                j2 * 256 + (qb + 1) * 128],
                                            qt[:, h,
                                               b * 256 + qb * 128:
                                               b * 256 + (qb + 1) * 128],
                                            start=True, stop=True)
                                e = esb.tile([128, SPG, 2, 256], FP8,
                                             name="e")
                                nc.scalar.activation(
                                    e[:], sp[:], AF.Exp,
                                    scale=cfg.Hd ** -0.5
                                    / (S_QK * S_QK if cfg.fp8_qk else 1.0),
                                    bias=negc[:])
                                if j2 == b:
                                    nc.vector.tensor_mul(
                                        e[:], e[:],
                                        mask_sb[:, SPG * pg:SPG * (pg + 1),
                                                :, :])
                                for g2 in range(SPG):
                                    nc.tensor.matmul(
                                        rp[:], ones8[:, :, :],
                                        e[:, g2, :, :],
                                        start=(pi == 0),
                                        stop=(pi == npairs - 1),
                                        perf_mode=DR)
                                    for qb in range(2):
                                        ck = j2 * n + 2 * SPG * pg + 2 * g2
                                        nc.tensor.matmul(
                                            otp[qb][:],
                                            vtb[qb][:, ck:ck + 2,
                                                    hi * 128:(hi + 1) * 128],
                                            e[:, g2, :,
                                              qb * 128:(qb + 1) * 128],
                                            start=(pi == 0),
                                            stop=(pi == npairs - 1),
                                            perf_mode=DR)
                                    pi += 1
                        for j2 in range(0 if cfg.fp8_att else b + 1):
                            for pm in range(NP2):
                                sp = ps_s.tile([128, 2, 256], F32, name="sp")
                                for ci in range(2):
                                    r = 2 * pm + ci
                                    for qb in range(2):
                                        nc.tensor.matmul(
                                            sp[:, ci,
                                               qb * 128:(qb + 1) * 128],
                                            ktb[:, r,
                                                j2 * 256 + qb * 128:
                                                j2 * 256 + (qb + 1) * 128],
                                            qt[:, h,
                                               b * 256 + qb * 128:
                                               b * 256 + (qb + 1) * 128],
                                            start=True, stop=True)
                                e = esb.tile([128, 2, 256], BF, name="e")
                                nc.scalar.activation(
                                    e[:], sp[:], AF.Exp,
                                    scale=cfg.Hd ** -0.5
                                    / (S_QK * S_QK if cfg.fp8_qk else 1.0))
                                if j2 == b:
                                    nc.vector.tensor_mul(
                                        e[:], e[:], mask_sb[:, pm, :, :])
                                if cfg.pool_rsum:
                                    # denominators: accumulate e on the (idle)
                                    # Pool engine; one ones-matmul at the end
                                    if pi == 0:
                                        esum = esump.tile([128, 2, 256], BF,
                                                          name="esum")
                                        nc.gpsimd.tensor_copy(esum[:], e[:])
                                    else:
                                        nc.gpsimd.tensor_add(esum[:],
                                                             esum[:], e[:])
                                else:
                                    nc.tensor.matmul(
                                        rp[:], ones[:, 0:1], e[:],
                                        start=(pi == 0),
                                        stop=(pi == npairs - 1))
                                for ci in range(2):
                                    r = 2 * pm + ci
                                    for qb in range(2):
                                        nc.tensor.matmul(
                                            otp[qb][:],
                                            vtb[qb][:, j2 * n + r,
                                                    hi * 128:(hi + 1) * 128],
                                            e[:, ci, qb * 128:(qb + 1) * 128],
                                            start=(pi == 0 and ci == 0),
                                            stop=(pi == npairs - 1
                                                  and ci == 1))
                                pi += 1
                        if cfg.pool_rsum and not cfg.fp8_att:
                            nc.tensor.matmul(rp[:], ones[:, 0:1], esum[:],
                                             start=True, stop=True)
                        if cfg.fp8_att:
                            # rp already holds r broadcast across partitions
                            rinv = aux.tile([128, 256], F32, name="rinv128")
                            nc.vector.reciprocal(rinv[:], rp[:])
                            if S_O != S_V:
                                nc.scalar.mul(rinv[:], rinv[:], S_O / S_V)
                            for qb in range(2):
                                nc.vector.tensor_mul(
                                    ot[:, h, b * 256 + qb * 128:
                                       b * 256 + (qb + 1) * 128],
                                    otp[qb][:],
                                    rinv[:, qb * 128:(qb + 1) * 128])
                            continue
                        rsum = aux.tile([1, 256], F32, name="rsum")
                        nc.vector.tensor_reduce(
                            rsum[:],
                            rp[:].rearrange("p (a q) -> p q a", a=2),
                            axis=mybir.AxisListType.X,
                            op=mybir.AluOpType.add)
                        rinv = aux.tile([1, 256], F32, name="rinv")
                        nc.vector.reciprocal(rinv[:], rsum[:])
                        rinv_b = aux.tile([1, 256], BF, name="rinv_b")
                        nc.scalar.mul(rinv_b[:], rinv[:],
                                      S_O if cfg.fp8_out else 1.0)
                        rbc = ps_b.tile([128, 256], F32, name="rbc")
                        nc.tensor.matmul(rbc[:], ones[0:1, :], rinv_b[:],
                                         start=True, stop=True)
                        rbc_sb = aux.tile([128, 256], BF, name="rbc_sb")
                        nc.scalar.copy(rbc_sb[:], rbc[:])
                        for qb in range(2):
                            nc.vector.tensor_mul(
                                ot[:, h,
                                   b * 256 + qb * 128:b * 256 + (qb + 1) * 128],
                                otp[qb][:], rbc_sb[:, qb * 128:(qb + 1) * 128])

        # ============== phase 5: out-proj + residual ==============
        # tb outer so each token block's x1 completes early for rmsnorm2
        with tc.tile_pool(name="ph5", bufs=2) as ph5, \
             tc.tile_pool(name="wo5", bufs=1) as wo5, \
             tc.tile_pool(name="ps_y", bufs=2, space="PSUM") as ps_y:
            wo_gs = []
            for ct in range(CT if SA >= 5 else 0):
                wo_g = wo5.tile([128, DC, CW], FP8 if cfg.fp8_out else BF,
                                name=f"wo_g{ct}")
                nc.sync.dma_start(wo_g[:], wout_io[ct, :, :, :])
                wo_gs.append(wo_g)
            for tb in range(TB if SA >= 5 else 0):
                for ct in range(CT):
                    c0 = ct * CW
                    wo_g = wo_gs[ct]
                    ps = ps_y.tile([128, CW], F32, name="ps_y")
                    if cfg.fp8_out:
                        for hc in range(0, H, 2):
                            nc.tensor.matmul(
                                ps[:],
                                ot[:, hc:hc + 2, tb * 128:(tb + 1) * 128],
                                wo_g[:, hc:hc + 2, :],
                                start=(hc == 0), stop=(hc == H - 2),
                                perf_mode=DR)
                        yv = ph5.tile([128, CW], BF, name="yv")
                        nc.scalar.mul(yv[:], ps[:], 1.0 / (S_W * S_O))
                        src = yv
                    else:
                        for hc in range(H):
                            nc.tensor.matmul(
                                ps[:], ot[:, hc, tb * 128:(tb + 1) * 128],
                                wo_g[:, hc, :],
                                start=(hc == 0), stop=(hc == H - 1))
                        src = ps
                    xr = ph5.tile([128, CW], F32, name="xr")
                    nc.sync.dma_start(
                        xr[:], x_io[tb * 128:(tb + 1) * 128, c0:c0 + CW])
                    nc.vector.tensor_add(x1_t[tb][:, c0:c0 + CW], src[:],
                                         xr[:])

        attn_stack.close()  # free qt / v_all / ot for the FFN hidden state
        if cfg.hh_sbuf:
            hhP = top.enter_context(tc.tile_pool(name="hhP", bufs=1))
            hh_sb = hhP.tile([128, FFB, T], BF, name="hh_sb")

        # ============== phase 6: rmsnorm2 + transpose ==============
        with tc.tile_pool(name="ph6", bufs=2) as ph6, \
             tc.tile_pool(name="ps_tp6", bufs=4, space="PSUM") as ps_tp6:
            if SA >= 6:
                rmsnorm_transpose(lambda tb: x1_t[tb], ps_tp6, ph6, xzt)

        # ============== phase 7: FFN up (w1/w3 + swiglu) ==============
        with tc.tile_pool(name="ph7", bufs=3) as ph7, \
             tc.tile_pool(name="w13", bufs=2) as w13p, \
             tc.tile_pool(name="ps_h", bufs=3, space="PSUM") as ps_h:
            for g in range(FFB // 2 if SA >= 7 else 0):  # 512-col groups
                wt = w13p.tile([128, DC, 512], BF, name="w13_t")
                nc.sync.dma_start(wt[:], w13_io[g, :, :, :])
                for fi in range(2):
                    f = 2 * g + fi
                    o1, o3 = fi * 256, fi * 256 + 128
                    h1 = ps_h.tile([128, T], F32, name="h1")
                    for dc in range(DC):
                        nc.tensor.matmul(h1[:], wt[:, dc, o1:o1 + 128],
                                         xzt[:, dc, :],
                                         start=(dc == 0), stop=(dc == DC - 1))
                    s1 = ph7.tile([128, T], BF, name="s1")
                    if cfg.use_silu:
                        nc.scalar.activation(s1[:], h1[:], AF.Silu)
                    else:
                        sg = ph7.tile([128, T], BF, name="sg")
                        nc.scalar.activation(sg[:], h1[:], AF.Sigmoid)
                        nc.vector.tensor_mul(s1[:], sg[:], h1[:])
                    h3 = ps_h.tile([128, T], F32, name="h3")
                    for dc in range(DC):
                        nc.tensor.matmul(h3[:], wt[:, dc, o3:o3 + 128],
                                         xzt[:, dc, :],
                                         start=(dc == 0), stop=(dc == DC - 1))
                    if cfg.hh_sbuf:
                        nc.vector.tensor_mul(hh_sb[:, f, :], s1[:], h3[:])
                    else:
                        hh = ph7.tile([128, T], BF, name="hh")
                        nc.vector.tensor_mul(hh[:], s1[:], h3[:])
                        nc.sync.dma_start(hh_d[f * 128:(f + 1) * 128, :],
                                          hh[:])

        # ============== phase 8: FFN down + residual + out ==============
        with tc.tile_pool(name="w2p", bufs=2) as w2p, \
             tc.tile_pool(name="hhp", bufs=3) as hhp, \
             tc.tile_pool(name="ps_y2", bufs=1, space="PSUM") as ps_y2, \
             tc.tile_pool(name="osb", bufs=2) as osb:
            per_pass = max(1, 8 // TB)  # col tiles per pass (8 psum banks)
            for p0 in range(0, CT if SA >= 8 else 0, per_pass):
                cts = list(range(p0, min(CT, p0 + per_pass)))
                pw = len(cts) * CW
                ps_t = {(tb, ct): ps_y2.tile([128, CW], F32,
                                             name=f"y2_{tb}_{ct - p0}")
                        for tb in range(TB) for ct in cts}
                for gf in range(FFB // W2G):
                    wt = w2p.tile([128, W2G, pw], BF, name="w2_t")
                    nc.sync.dma_start(
                        wt[:], w2_io[gf, :, :, p0 * CW:p0 * CW + pw])
                    for fi in range(W2G):
                        fc = gf * W2G + fi
                        if cfg.hh_sbuf:
                            def hh_s(tb, fc=fc):
                                return hh_sb[:, fc,
                                             tb * 128:(tb + 1) * 128]
                        else:
                            hht = hhp.tile([128, T], BF, name="hh_s")
                            nc.sync.dma_start(
                                hht[:], hh_d[fc * 128:(fc + 1) * 128, :])

                            def hh_s(tb, hht=hht):
                                return hht[:, tb * 128:(tb + 1) * 128]
                        for tb in range(TB):
                            for ct in cts:
                                o = (ct - p0) * CW
                                nc.tensor.matmul(
                                    ps_t[(tb, ct)][:],
                                    hh_s(tb),
                                    wt[:, fi, o:o + CW],
                                    start=(fc == 0), stop=(fc == FFB - 1))
                for tb in range(TB):
                    for ct in cts:
                        c0 = ct * CW
                        o = osb.tile([128, CW], F32, name="o_sb")
                        nc.vector.tensor_add(o[:], ps_t[(tb, ct)][:],
                                             x1_t[tb][:, c0:c0 + CW])
                        nc.sync.dma_start(
                            out_io[tb * 128:(tb + 1) * 128, c0:c0 + CW], o[:])

        if SA < 8:  # truncated probe build: emit dummy output
            for tb in range(TB):
                nc.any.memset(x1_t[tb][:, 0:1], 0.0)
                nc.sync.dma_start(out_io[tb * 128:(tb + 1) * 128, :],
                                  x1_t[tb][:])

    nc.compile()
    return nc


# --------------------------- host-side prep ---------------------------

def host_prep(cfg: Cfg, x, w_qkv, w_out, w1, w2, w3, g1, g2):
    """Build the per-core input maps (numpy, bf16 weights, mask data)."""
    n, D, H, DFF = cfg.n_cores, cfg.D, cfg.H, cfg.DFF
    S_blk, DC, NK, QB, FFB = cfg.S_blk, cfg.DC, cfg.NK, cfg.QB, cfg.FFB
    GW, CW, CT, W2G, T = cfg.GW, cfg.CW, cfg.CT, cfg.W2G, cfg.T

    def group_layout(w, gw):
        # [D, C] -> [C//gw, 128, DC, gw]
        C = w.shape[1]
        return np.ascontiguousarray(
            w.reshape(DC, 128, C // gw, gw).transpose(2, 1, 0, 3))

    x = np.asarray(x, np.float32)
    g1 = np.asarray(g1, np.float32)
    g2 = np.asarray(g2, np.float32)

    # softmax scale Hd^-0.5 is applied in the kernel's exp (not folded here)
    wqkv = np.asarray(w_qkv, np.float32) * g1[:, None]
    if cfg.fp8_qkv:
        wqkv_t = np.clip(group_layout(wqkv, GW) * S_W,
                         -240.0, 240.0).astype(E4M3)
    else:
        wqkv_t = group_layout(wqkv, GW).astype(BF16)

    wout = np.asarray(w_out, np.float32)
    if cfg.fp8_out:
        wout_t = np.clip(group_layout(wout, CW) * S_W,
                         -240.0, 240.0).astype(E4M3)
    else:
        wout_t = group_layout(wout, CW).astype(BF16)

    w1g = (np.asarray(w1, np.float32) * g2[:, None]).reshape(DC, 128, FFB, 128)
    w3g = (np.asarray(w3, np.float32) * g2[:, None]).reshape(DC, 128, FFB, 128)
    w13 = np.stack([w1g, w3g], axis=3).reshape(DC, 128, 2 * DFF)
    w13_t = group_layout(w13.reshape(DC * 128, 2 * DFF), 512).astype(BF16)

    w2_t = np.ascontiguousarray(
        np.asarray(w2, np.float32).reshape(FFB // W2G, W2G, 128, D)
        .transpose(0, 2, 1, 3)).astype(BF16)

    ident = np.eye(128, dtype=np.float32).astype(BF16)
    ones128 = np.ones((128, 128), np.float32).astype(BF16)

    # local order: l = blk*256 + batch*128 + i ; token = n*(blk*128+i) + c
    l = np.arange(cfg.T)
    blk, qb_a, i_a = l // 256, (l // 128) % 2, l % 128
    base_pos = n * (blk * 128 + i_a)

    in_maps = []
    for c in range(n):
        pos = base_pos + c
        x_c = np.ascontiguousarray(x[qb_a, pos, :])
        # diagonal-chunk masks: key (p, rank r) vs query (q, core c):
        # allowed iff p < q or (p == q and r <= c)
        masks = np.zeros((128, n // 2, 2, 256), np.float32)
        kp = np.arange(128)[:, None]
        qq = np.arange(128)[None, :]
        for r in range(n):
            m = (kp < qq) | ((kp == qq) & (r <= c))
            masks[:, r // 2, r % 2, 0:128] = m
            masks[:, r // 2, r % 2, 128:256] = m
        in_maps.append({
            "x_c": x_c,
            "wqkv_t": wqkv_t, "wout_t": wout_t, "w13_t": w13_t, "w2_t": w2_t,
            "masks": masks.astype(E4M3 if cfg.fp8_att else BF16),
            "ident": ident, "ones128": ones128,
        })
    return in_maps


def assemble(cfg: Cfg, results):
    n = cfg.n_cores
    out = np.empty((cfg.B, cfg.S, cfg.D), np.float32)
    l = np.arange(cfg.T)
    blk, qb_a, i_a = l // 256, (l // 128) % 2, l % 128
    base_pos = n * (blk * 128 + i_a)
    for c in range(n):
        out[qb_a, base_pos + c] = results[c]["out_c"]
    return out


_NC_CACHE = {}


def get_nc(cfg: Cfg = FULL):
    if cfg not in _NC_CACHE:
        _NC_CACHE[cfg] = build_nc(cfg)
    return _NC_CACHE[cfg]


def kernel(x, w_qkv, w_out, w1, w2, w3, g1, g2):
    cfg = FULL
    nc = get_nc(cfg)
    in_maps = host_prep(cfg, x, w_qkv, w_out, w1, w2, w3, g1, g2)
    res = run_bass_kernel_spmd(nc, in_maps, core_ids=list(range(cfg.n_cores)))
    return assemble(cfg, res.results)



# revision 73
# speedup vs baseline: 1.1811x; 1.1811x over previous
"""Trainium2 Bass kernel for a dense transformer block (RMSNorm -> causal MHA
-> residual -> RMSNorm -> SwiGLU FFN -> residual).

Sharding: data-parallel over tokens with a *strided* assignment -- core c
owns every token position == c (mod 8) of both batches (512 tokens/core),
weights replicated. The stride makes the causal chunk structure identical on
every core (no padding waste), so one SPMD program serves all cores; the
residual per-core causality lands in small per-core mask *data* for the
diagonal key chunks only. K/V are exchanged with one AllGather pair.

Matmuls run in bf16 with fp32 PSUM accumulation; residuals/norms stay fp32.
Attention processes chunk pairs for both batches at once ([128, 512] exp and
mask tiles), softmax denominators accumulate on the PE via a ones-column
matmul, and 1/r normalization is applied via a PE partition-broadcast.
"""

from dataclasses import dataclass
from contextlib import ExitStack

import numpy as np

import concourse.bacc as bacc
import concourse.mybir as mybir
import concourse.tile as tile
from concourse.bass_utils import run_bass_kernel_spmd

try:
    import ml_dtypes

    BF16 = ml_dtypes.bfloat16
    E4M3 = ml_dtypes.float8_e4m3  # TRN fp8e4: max +-240, matches <=240
except ImportError:  # pragma: no cover
    import jax.numpy as jnp

    BF16 = jnp.bfloat16
    E4M3 = jnp.float8_e4m3

F32 = mybir.dt.float32
BF = mybir.dt.bfloat16
FP8 = mybir.dt.float8e4
AF = mybir.ActivationFunctionType
DR = mybir.MatmulPerfMode.DoubleRow

# fp8 scaling: weights/activations are pre-scaled out of the subnormal range
# (w sigma ~ 1/sqrt(D), xn ~ unit RMS); descale on PSUM copy-out.
S_W = 128.0  # qkv / out weight scale
S_X = 16.0   # normalized-activation scale
S_O = 16.0   # attention-output scale
S_V = 16.0   # V scale for fp8 attention weights
S_QK = 16.0  # Q/K scale for fp8 scores
C_SHIFT = 3.0  # exp shift: e' = exp(s/sqrt(Hd) - C); cancels in softmax


@dataclass(frozen=True)
class Cfg:
    B: int = 2
    S: int = 2048
    D: int = 2048
    H: int = 16
    DFF: int = 8192
    n_cores: int = 8
    eps: float = 1e-6
    use_silu: bool = False  # ACT Silu table vs sigmoid+mul (sim lacks Silu)
    fake_ag: bool = False   # replace AllGather with local DMA (TimelineSim)
    fp8_qkv: bool = True    # qkv projection in fp8e4 DoubleRow (2x PE)
    fp8_out: bool = True    # attention out-projection in fp8e4 DoubleRow
    pool_rsum: bool = False  # softmax denominators: accumulate e on Pool
    fp8_att: bool = True    # shifted-exp fp8 attn weights + fp8 V (DR av/rsum)
    fp8_qk: bool = True     # fp8 Q/K (fp8 scores; K travels fp8 on the wire)
    merged_ag: bool = False  # single AllGather carrying K and V (slower:
    #   2MB/rank leaves the <1MB low-latency mesh collective regime)
    hh_sbuf: bool = True    # keep FFN hidden activations SBUF-resident
    stop_after: int = 0     # timing probe: truncate after phase N (0 = full)

    @property
    def Hd(self):
        return self.D // self.H

    @property
    def S_blk(self):
        return self.S // self.n_cores  # per-(core,batch) query block

    @property
    def T(self):
        return self.B * self.S_blk  # tokens per core

    @property
    def TB(self):
        return self.T // 128

    @property
    def DC(self):
        return self.D // 128

    @property
    def NK(self):
        return self.S // 128  # padded key chunks per batch

    @property
    def QB(self):
        return self.S_blk

    @property
    def FFB(self):
        return self.DFF // 128

    @property
    def CT(self):
        return max(1, self.D // 512)

    @property
    def CW(self):
        return min(512, self.D)

    @property
    def GW(self):
        return min(512, self.D)  # qkv weight col group width

    @property
    def W2G(self):
        return 4  # w2 chunks per DMA group


FULL = Cfg()


def build_nc(cfg: Cfg):
    """Build the per-core Bass program (identical on all cores)."""
    assert cfg.Hd == 128 and cfg.S_blk % 128 == 0 and cfg.T % 128 == 0
    n, D, H, T, DC, NK, QB, FFB = (
        cfg.n_cores, cfg.D, cfg.H, cfg.T, cfg.DC, cfg.NK, cfg.QB, cfg.FFB)
    S_blk, TB, CT, CW, DFF, GW, W2G = (
        cfg.S_blk, cfg.TB, cfg.CT, cfg.CW, cfg.DFF, cfg.GW, cfg.W2G)
    SB2 = S_blk // 128
    assert FFB % W2G == 0 and (3 * D) % GW == 0
    assert n % 4 == 0 and T % 256 == 0

    nc = bacc.Bacc("TRN2", target_bir_lowering=False, debug=False,
                   num_devices=n)

    # ---- I/O (host pre-tiled layouts; one DMA per weight group) ----
    x_io = nc.dram_tensor("x_c", [T, D], F32, kind="ExternalInput")
    wqkv_io = nc.dram_tensor("wqkv_t", [3 * D // GW, 128, DC, GW],
                             FP8 if cfg.fp8_qkv else BF,
                             kind="ExternalInput")
    wout_io = nc.dram_tensor("wout_t", [CT, 128, DC, CW],
                             FP8 if cfg.fp8_out else BF,
                             kind="ExternalInput")
    w13_io = nc.dram_tensor("w13_t", [FFB // 2, 128, DC, 512], BF,
                            kind="ExternalInput")
    w2_io = nc.dram_tensor("w2_t", [FFB // W2G, 128, W2G, D], BF,
                           kind="ExternalInput")
    masks_io = nc.dram_tensor("masks", [128, n // 2, 2, 256],
                              FP8 if cfg.fp8_att else BF,
                              kind="ExternalInput")
    ident_io = nc.dram_tensor("ident", [128, 128], BF, kind="ExternalInput")
    ones_io = nc.dram_tensor("ones128", [128, 128], BF, kind="ExternalInput")
    out_io = nc.dram_tensor("out_c", [T, D], F32, kind="ExternalOutput")

    SA = cfg.stop_after if cfg.stop_after else 99  # phase truncation probe
    with tile.TileContext(nc) as tc, ExitStack() as top:
        P = top.enter_context(tc.tile_pool(name="persist", bufs=1))
        consts = top.enter_context(tc.tile_pool(name="consts", bufs=1))

        ident = consts.tile([128, 128], BF, name="ident_sb")
        ones = consts.tile([128, 128], BF, name="ones_sb")
        mask_sb = consts.tile([128, n // 2, 2, 256],
                              FP8 if cfg.fp8_att else BF, name="mask_sb")
        eps_t = consts.tile([128, 1], F32, name="eps_sb")
        nc.sync.dma_start(ident[:], ident_io[:, :])
        nc.any.memset(eps_t[:], cfg.eps)
        if cfg.fp8_att:
            # all-ones fp8 weights, 128 out-columns: the denominator matmul
            # rp = ones8^T @ e lands broadcast across all 128 partitions
            ones8 = consts.tile([128, 2, 128], FP8, name="ones8_sb")
            nc.any.memset(ones8[:], 1.0)
            negc = consts.tile([128, 1], F32, name="negc_sb")
            nc.any.memset(negc[:], -C_SHIFT)

        x1_t = [P.tile([128, D], F32, name=f"x1_{tb}") for tb in range(TB)]
        xzt = P.tile([128, DC, T], BF, name="xzt")       # z2^T for FFN
        xzt8 = (P.tile([128, DC, T], FP8, name="xzt8")   # xn^T fp8 for QKV
                if cfg.fp8_qkv else xzt)

        # attention-scoped persistents: freed before the FFN phases so the
        # SwiGLU hidden state can live in SBUF
        attn_stack = ExitStack()
        AP = attn_stack.enter_context(tc.tile_pool(name="attnP", bufs=1))
        QKDT = FP8 if cfg.fp8_qk else BF
        qt = AP.tile([128, H, T], QKDT, name="qt")       # Q d-major
        v_all = AP.tile([128, TB, D], FP8 if cfg.fp8_att else BF,
                        name="v_all")                    # V token-major
        ot = AP.tile([128, H, T], FP8 if cfg.fp8_out else BF,
                     name="ot")                          # attn out d-major

        dram = top.enter_context(tc.tile_pool(name="dram", bufs=1, space="DRAM"))
        VDT = FP8 if cfg.fp8_att else BF
        if cfg.merged_ag:
            assert cfg.fp8_qk and cfg.fp8_att
            # fp8 [4096, 512]: rows [0, D) = K^T; rows [D, 2D) = V flattened
            # (token t = 4 rows of 512 features each)
            kv_c = dram.tile([2 * D, T], FP8, name="kv_contrib")
            kv_g = dram.tile([n * 2 * D, T], FP8, name="kv_gath",
                             addr_space="Shared")
        else:
            kt_c = dram.tile([D, T], QKDT, name="kt_contrib")
            v_c = dram.tile([T, D], VDT, name="v_contrib")
            kt_g = dram.tile([n * D, T], QKDT, name="kt_gath",
                             addr_space="Shared")
            v_g = dram.tile([n * T, D], VDT, name="v_gath",
                            addr_space="Shared")
        if not cfg.hh_sbuf:
            hh_d = dram.tile([DFF, T], BF, name="hh_d")  # swiglu spill

        def rmsnorm_transpose(get_src, ps_tp, pool, dst, out_scale=1.0):
            """token-major fp32 tiles -> d-major bf16/fp8 into dst."""
            for tb in range(TB):
                xt = get_src(tb)
                scr = pool.tile([128, D], BF, name="nrm_scr")
                ssq = pool.tile([128, 1], F32, name="nrm_ssq")
                nc.scalar.activation(scr[:], xt[:], AF.Square,
                                     accum_out=ssq[:])
                sd = pool.tile([128, 1], F32, name="nrm_sd")
                nc.scalar.activation(sd[:], ssq[:], AF.Sqrt,
                                     bias=eps_t[:], scale=1.0 / D)
                inv = pool.tile([128, 1], F32, name="nrm_inv")
                nc.vector.reciprocal(inv[:], sd[:])
                xn = pool.tile([128, D], BF, name="nrm_xn")
                nc.vector.tensor_scalar_mul(xn[:], xt[:], inv[:])
                for dc in range(DC):
                    tp = ps_tp.tile([128, 128], BF, name="tp")
                    nc.tensor.transpose(tp[:], xn[:, dc * 128:(dc + 1) * 128],
                                        ident[:])
                    d = dst[:, dc, tb * 128:(tb + 1) * 128]
                    if out_scale != 1.0:
                        nc.scalar.mul(d, tp[:], out_scale)
                    else:
                        nc.vector.tensor_copy(d, tp[:])

        # ================= phase 1: rmsnorm1 + transpose =================
        with tc.tile_pool(name="ph1", bufs=2) as ph1, \
             tc.tile_pool(name="ps_tp1", bufs=4, space="PSUM") as ps_tp1:

            def load_x(tb):
                t = ph1.tile([128, D], F32, name=f"xld_{tb}")
                nc.sync.dma_start(t[:], x_io[tb * 128:(tb + 1) * 128, :])
                return t

            rmsnorm_transpose(load_x, ps_tp1, ph1, xzt8,
                              S_X if cfg.fp8_qkv else 1.0)
            # consts not needed until attention -- keep them off the DMA
            # queue ahead of the x loads
            nc.sync.dma_start(ones[:], ones_io[:, :])
            nc.sync.dma_start(mask_sb[:], masks_io[:, :, :, :])

        # ============== phase 2: qkv projections + AllGather ==============
        # order: K, V (feed the AllGather), then Q (overlaps the AG)
        with tc.tile_pool(name="ph2", bufs=1) as ph2, \
             tc.tile_pool(name="wg", bufs=3) as wg, \
             tc.tile_pool(name="ps_mm", bufs=3, space="PSUM") as ps_mm:
            kt_l = ph2.tile([128, H, T], QKDT, name="kt_l")  # local K d-major
            qkscale = ((S_QK if cfg.fp8_qk else 1.0)
                       / ((S_W * S_X) if cfg.fp8_qkv else 1.0))

            def load_group(g):
                t = wg.tile([128, DC, GW], FP8 if cfg.fp8_qkv else BF,
                            name="wg_t")
                nc.sync.dma_start(t[:], wqkv_io[g, :, :, :])
                return t

            def qk_block(wt, off, dst, slot):
                ps = ps_mm.tile([128, T], F32, name="ps_qk")
                if cfg.fp8_qkv:
                    for dc in range(0, DC, 2):
                        nc.tensor.matmul(ps[:], wt[:, dc:dc + 2, off:off + 128],
                                         xzt8[:, dc:dc + 2, :],
                                         start=(dc == 0), stop=(dc == DC - 2),
                                         perf_mode=DR)
                else:
                    for dc in range(DC):
                        nc.tensor.matmul(ps[:], wt[:, dc, off:off + 128],
                                         xzt[:, dc, :],
                                         start=(dc == 0), stop=(dc == DC - 1))
                nc.scalar.mul(dst[:, slot, :], ps[:], qkscale)

            # K: wqkv cols [D, 2D)
            for g in range(D // GW, 2 * D // GW if SA >= 2 else 0):
                wt = load_group(g)
                for b in range(GW // 128):
                    h = g * GW // 128 + b - H
                    qk_block(wt, b * 128, kt_l, h)
            # V: wqkv cols [2D, 3D) -> token-major
            for g in range(2 * D // GW, 3 * D // GW if SA >= 2 else 0):
                wt = load_group(g)
                base = g * GW - 2 * D
                vscale = ((S_V if cfg.fp8_att else 1.0)
                          / ((S_W * S_X) if cfg.fp8_qkv else 1.0))
                for tb in range(TB):
                    for c0 in range(0, GW, CW):
                        ps = ps_mm.tile([128, CW], F32, name="ps_v")
                        if cfg.fp8_qkv:
                            for dc in range(0, DC, 2):
                                nc.tensor.matmul(
                                    ps[:],
                                    xzt8[:, dc:dc + 2, tb * 128:(tb + 1) * 128],
                                    wt[:, dc:dc + 2, c0:c0 + CW],
                                    start=(dc == 0), stop=(dc == DC - 2),
                                    perf_mode=DR)
                        else:
                            for dc in range(DC):
                                nc.tensor.matmul(
                                    ps[:], xzt[:, dc, tb * 128:(tb + 1) * 128],
                                    wt[:, dc, c0:c0 + CW],
                                    start=(dc == 0), stop=(dc == DC - 1))
                        nc.scalar.mul(
                            v_all[:, tb, base + c0:base + c0 + CW], ps[:],
                            vscale)

            for h in range(H if SA >= 2 else 0):
                dst = kv_c if cfg.merged_ag else kt_c
                nc.sync.dma_start(dst[h * 128:(h + 1) * 128, :],
                                  kt_l[:, h, :])
            for tb in range(TB if SA >= 2 else 0):
                if cfg.merged_ag:
                    nc.sync.dma_start(
                        kv_c[D + tb * 512:D + (tb + 1) * 512, :]
                        .rearrange("(p a) q -> p a q", a=4),
                        v_all[:, tb, :]
                        .rearrange("p (a q) -> p a q", a=4))
                else:
                    nc.sync.dma_start(v_c[tb * 128:(tb + 1) * 128, :],
                                      v_all[:, tb, :])
            if SA < 2:
                pass
            elif cfg.fake_ag:
                if cfg.merged_ag:
                    nc.sync.dma_start(kv_g[0:2 * D, :], kv_c[:, :])
                else:
                    nc.sync.dma_start(kt_g[0:D, :], kt_c[:, :])
                    nc.sync.dma_start(v_g[0:T, :], v_c[:, :])
            elif cfg.merged_ag:
                nc.gpsimd.collective_compute(
                    "AllGather", mybir.AluOpType.bypass,
                    replica_groups=[list(range(n))],
                    ins=[kv_c.opt()], outs=[kv_g.opt()])
            else:
                nc.gpsimd.collective_compute(
                    "AllGather", mybir.AluOpType.bypass,
                    replica_groups=[list(range(n))],
                    ins=[kt_c.opt()], outs=[kt_g.opt()])
                nc.gpsimd.collective_compute(
                    "AllGather", mybir.AluOpType.bypass,
                    replica_groups=[list(range(n))],
                    ins=[v_c.opt()], outs=[v_g.opt()])

            # Q: wqkv cols [0, D)  (overlaps the AG)
            for g in range(0, D // GW if SA >= 2 else 0):
                wt = load_group(g)
                for b in range(GW // 128):
                    h = g * GW // 128 + b
                    qk_block(wt, b * 128, qt, h)

        # ================= phase 4: attention =================
        # Strided token assignment: core c owns tokens == c (mod n); local
        # order is l = blk*256 + batch*128 + i  (token = n*(blk*128+i) + c).
        # Every local 256-query super-block b needs exactly (b+1)*n key
        # chunks on every core -- no padding. Only diagonal (j2 == b) chunks
        # need a mask; mask content is per-core data. Chunks are processed
        # in pairs so exp/mask run on [128, 512] tiles.
        HG = min(4, H)
        NP2 = n // 2
        SPG = 2 if cfg.fp8_att else 1  # score pairs fused per exp call
        with tc.tile_pool(name="kv", bufs=2) as kv, \
             tc.tile_pool(name="ktp", bufs=3) as ktp, \
             tc.tile_pool(name="esb", bufs=6 // SPG) as esb, \
             tc.tile_pool(name="esump", bufs=2) as esump, \
             tc.tile_pool(name="aux", bufs=2) as aux, \
             tc.tile_pool(name="ps_s", bufs=4 // SPG, space="PSUM") as ps_s, \
             tc.tile_pool(name="ps_ot", bufs=1, space="PSUM") as ps_ot, \
             tc.tile_pool(name="ps_r", bufs=1, space="PSUM") as ps_r, \
             tc.tile_pool(name="ps_b", bufs=1, space="PSUM") as ps_b:
            for hq in range(H // HG if SA >= 4 else 0):
                vtb = [kv.tile([128, NK, HG * 128],
                               FP8 if cfg.fp8_att else BF, name=f"vtb{qb}")
                       for qb in range(2)]
                for qb in range(2):
                    for r in range(n):
                        for j2 in range(SB2):
                            t0 = j2 * 256 + qb * 128
                            if cfg.merged_ag:
                                base = r * 2 * D + D
                                src = (kv_g[base + 4 * t0:
                                            base + 4 * (t0 + 128), :]
                                       .rearrange("(p a) q -> p a q", a=4)
                                       [:, hq, :])
                            else:
                                src = v_g[r * T + t0:r * T + t0 + 128,
                                          hq * HG * 128:(hq + 1) * HG * 128]
                            nc.sync.dma_start(vtb[qb][:, j2 * n + r, :], src)
                for hi in range(HG):
                    h = hq * HG + hi
                    ktb = ktp.tile([128, n, T], QKDT, name="ktb")
                    for r in range(n):
                        if cfg.merged_ag:
                            src = kv_g[r * 2 * D + h * 128:
                                       r * 2 * D + (h + 1) * 128, :]
                        else:
                            src = kt_g[r * D + h * 128:r * D + (h + 1) * 128,
                                       :]
                        nc.sync.dma_start(ktb[:, r, :], src)
                    for b in range(SB2):
                        otp = [ps_ot.tile([128, 128], F32, name=f"otp{qb}")
                               for qb in range(2)]
                        rp = ps_r.tile(
                            [128, 256] if cfg.fp8_att else [1, 512], F32,
                            name="rp")
                        npairs = (b + 1) * NP2
                        pi = 0
                        for j2 in range(b + 1 if cfg.fp8_att else 0):
                            # fp8 path: 2 pairs (4 key chunks) per exp/mask
                            for pg in range(NP2 // SPG):
                                sp = ps_s.tile([128, SPG, 2, 256], F32,
                                               name="sp")
                                for c in range(2 * SPG):
                                    r = 2 * SPG * pg + c
                                    for qb in range(2):
                                        nc.tensor.matmul(
                                            sp[:, c // 2, c % 2,
                                               qb * 128:(qb + 1) * 128],
                                            ktb[:, r,
                                                j2 * 256 + qb * 128:
                                                j2 * 256 + (qb + 1) * 128],
                                            qt[:, h,
                                               b * 256 + qb * 128:
                                               b * 256 + (qb + 1) * 128],
                                            start=True, stop=True)
                                e = esb.tile([128, SPG, 2, 256], FP8,
                                             name="e")
                                nc.scalar.activation(
                                    e[:], sp[:], AF.Exp,
                                    scale=cfg.Hd ** -0.5
                                    / (S_QK * S_QK if cfg.fp8_qk else 1.0),
                                    bias=negc[:])
                                if j2 == b:
                                    nc.vector.tensor_mul(
                                        e[:], e[:],
                                        mask_sb[:, SPG * pg:SPG * (pg + 1),
                                                :, :])
                                for g2 in range(SPG):
                                    nc.tensor.matmul(
                                        rp[:], ones8[:, :, :],
                                        e[:, g2, :, :],
                                        start=(pi == 0),
                                        stop=(pi == npairs - 1),
                                        perf_mode=DR)
                                    for qb in range(2):
                                        ck = j2 * n + 2 * SPG * pg + 2 * g2
                                        nc.tensor.matmul(
                                            otp[qb][:],
                                            vtb[qb][:, ck:ck + 2,
                                                    hi * 128:(hi + 1) * 128],
                                            e[:, g2, :,
                                              qb * 128:(qb + 1) * 128],
                                            start=(pi == 0),
                                            stop=(pi == npairs - 1),
                                            perf_mode=DR)
                                    pi += 1
                        for j2 in range(0 if cfg.fp8_att else b + 1):
                            for pm in range(NP2):
                                sp = ps_s.tile([128, 2, 256], F32, name="sp")
                                for ci in range(2):
                                    r = 2 * pm + ci
                                    for qb in range(2):
                                        nc.tensor.matmul(
                                            sp[:, ci,
                                               qb * 128:(qb + 1) * 128],
                                            ktb[:, r,
                                                j2 * 256 + qb * 128:
                                                j2 * 256 + (qb + 1) * 128],
                                            qt[:, h,
                                               b * 256 + qb * 128:
                                               b * 256 + (qb + 1) * 128],
                                            start=True, stop=True)
                                e = esb.tile([128, 2, 256], BF, name="e")
                                nc.scalar.activation(
                                    e[:], sp[:], AF.Exp,
                                    scale=cfg.Hd ** -0.5
                                    / (S_QK * S_QK if cfg.fp8_qk else 1.0))
                                if j2 == b:
                                    nc.vector.tensor_mul(
                                        e[:], e[:], mask_sb[:, pm, :, :])
                                if cfg.pool_rsum:
                                    # denominators: accumulate e on the (idle)
                                    # Pool engine; one ones-matmul at the end
                                    if pi == 0:
                                        esum = esump.tile([128, 2, 256], BF,
                                                          name="esum")
                                        nc.gpsimd.tensor_copy(esum[:], e[:])
                                    else:
                                        nc.gpsimd.tensor_add(esum[:],
                                                             esum[:], e[:])
                                else:
                                    nc.tensor.matmul(
                                        rp[:], ones[:, 0:1], e[:],
                                        start=(pi == 0),
                                        stop=(pi == npairs - 1))
                                for ci in range(2):
                                    r = 2 * pm + ci
                                    for qb in range(2):
                                        nc.tensor.matmul(
                                            otp[qb][:],
                                            vtb[qb][:, j2 * n + r,
                                                    hi * 128:(hi + 1) * 128],
                                            e[:, ci, qb * 128:(qb + 1) * 128],
                                            start=(pi == 0 and ci == 0),
                                            stop=(pi == npairs - 1
                                                  and ci == 1))
                                pi += 1
                        if cfg.pool_rsum and not cfg.fp8_att:
                            nc.tensor.matmul(rp[:], ones[:, 0:1], esum[:],
                                             start=True, stop=True)
                        if cfg.fp8_att:
                            # rp already holds r broadcast across partitions
                            rinv = aux.tile([128, 256], F32, name="rinv128")
                            nc.vector.reciprocal(rinv[:], rp[:])
                            if S_O != S_V:
                                nc.scalar.mul(rinv[:], rinv[:], S_O / S_V)
                            for qb in range(2):
                                nc.vector.tensor_mul(
                                    ot[:, h, b * 256 + qb * 128:
                                       b * 256 + (qb + 1) * 128],
                                    otp[qb][:],
                                    rinv[:, qb * 128:(qb + 1) * 128])
                            continue
                        rsum = aux.tile([1, 256], F32, name="rsum")
                        nc.vector.tensor_reduce(
                            rsum[:],
                            rp[:].rearrange("p (a q) -> p q a", a=2),
                            axis=mybir.AxisListType.X,
                            op=mybir.AluOpType.add)
                        rinv = aux.tile([1, 256], F32, name="rinv")
                        nc.vector.reciprocal(rinv[:], rsum[:])
                        rinv_b = aux.tile([1, 256], BF, name="rinv_b")
                        nc.scalar.mul(rinv_b[:], rinv[:],
                                      S_O if cfg.fp8_out else 1.0)
                        rbc = ps_b.tile([128, 256], F32, name="rbc")
                        nc.tensor.matmul(rbc[:], ones[0:1, :], rinv_b[:],
                                         start=True, stop=True)
                        rbc_sb = aux.tile([128, 256], BF, name="rbc_sb")
                        nc.scalar.copy(rbc_sb[:], rbc[:])
                        for qb in range(2):
                            nc.vector.tensor_mul(
                                ot[:, h,
                                   b * 256 + qb * 128:b * 256 + (qb + 1) * 128],
                                otp[qb][:], rbc_sb[:, qb * 128:(qb + 1) * 128])

        # ============== phase 5: out-proj + residual ==============
        # tb outer so each token block's x1 completes early for rmsnorm2
        with tc.tile_pool(name="ph5", bufs=2) as ph5, \
             tc.tile_pool(name="wo5", bufs=1) as wo5, \
             tc.tile_pool(name="ps_y", bufs=2, space="PSUM") as ps_y:
            wo_gs = []
            for ct in range(CT if SA >= 5 else 0):
                wo_g = wo5.tile([128, DC, CW], FP8 if cfg.fp8_out else BF,
                                name=f"wo_g{ct}")
                nc.sync.dma_start(wo_g[:], wout_io[ct, :, :, :])
                wo_gs.append(wo_g)
            for tb in range(TB if SA >= 5 else 0):
                for ct in range(CT):
                    c0 = ct * CW
                    wo_g = wo_gs[ct]
                    ps = ps_y.tile([128, CW], F32, name="ps_y")
                    if cfg.fp8_out:
                        for hc in range(0, H, 2):
                            nc.tensor.matmul(
                                ps[:],
                                ot[:, hc:hc + 2, tb * 128:(tb + 1) * 128],
                                wo_g[:, hc:hc + 2, :],
                                start=(hc == 0), stop=(hc == H - 2),
                                perf_mode=DR)
                        yv = ph5.tile([128, CW], BF, name="yv")
                        nc.scalar.mul(yv[:], ps[:], 1.0 / (S_W * S_O))
                        src = yv
                    else:
                        for hc in range(H):
                            nc.tensor.matmul(
                                ps[:], ot[:, hc, tb * 128:(tb + 1) * 128],
                                wo_g[:, hc, :],
                                start=(hc == 0), stop=(hc == H - 1))
                        src = ps
                    xr = ph5.tile([128, CW], F32, name="xr")
                    nc.sync.dma_start(
                        xr[:], x_io[tb * 128:(tb + 1) * 128, c0:c0 + CW])
                    nc.vector.tensor_add(x1_t[tb][:, c0:c0 + CW], src[:],
                                         xr[:])

        attn_stack.close()  # free qt / v_all / ot for the FFN hidden state
        if cfg.hh_sbuf:
            hhP = top.enter_context(tc.tile_pool(name="hhP", bufs=1))
            hh_sb = hhP.tile([128, FFB, T], BF, name="hh_sb")

        # ============== phase 6: rmsnorm2 + transpose ==============
        with tc.tile_pool(name="ph6", bufs=2) as ph6, \
             tc.tile_pool(name="ps_tp6", bufs=4, space="PSUM") as ps_tp6:
            if SA >= 6:
                rmsnorm_transpose(lambda tb: x1_t[tb], ps_tp6, ph6, xzt)

        # ============== phase 7: FFN up (w1/w3 + swiglu) ==============
        with tc.tile_pool(name="ph7", bufs=3) as ph7, \
             tc.tile_pool(name="w13", bufs=2) as w13p, \
             tc.tile_pool(name="ps_h", bufs=3, space="PSUM") as ps_h:
            for g in range(FFB // 2 if SA >= 7 else 0):  # 512-col groups
                wt = w13p.tile([128, DC, 512], BF, name="w13_t")
                nc.sync.dma_start(wt[:], w13_io[g, :, :, :])
                for fi in range(2):
                    f = 2 * g + fi
                    o1, o3 = fi * 256, fi * 256 + 128
                    h1 = ps_h.tile([128, T], F32, name="h1")
                    for dc in range(DC):
                        nc.tensor.matmul(h1[:], wt[:, dc, o1:o1 + 128],
                                         xzt[:, dc, :],
                                         start=(dc == 0), stop=(dc == DC - 1))
                    s1 = ph7.tile([128, T], BF, name="s1")
                    if cfg.use_silu:
                        nc.scalar.activation(s1[:], h1[:], AF.Silu)
                    else:
                        sg = ph7.tile([128, T], BF, name="sg")
                        nc.scalar.activation(sg[:], h1[:], AF.Sigmoid)
                        nc.vector.tensor_mul(s1[:], sg[:], h1[:])
                    h3 = ps_h.tile([128, T], F32, name="h3")
                    for dc in range(DC):
                        nc.tensor.matmul(h3[:], wt[:, dc, o3:o3 + 128],
                                         xzt[:, dc, :],
                                         start=(dc == 0), stop=(dc == DC - 1))
                    if cfg.hh_sbuf:
                        nc.vector.tensor_mul(hh_sb[:, f, :], s1[:], h3[:])
                    else:
                        hh = ph7.tile([128, T], BF, name="hh")
                        nc.vector.tensor_mul(hh[:], s1[:], h3[:])
                        nc.sync.dma_start(hh_d[f * 128:(f + 1) * 128, :],
                                          hh[:])

        # ============== phase 8: FFN down + residual + out ==============
        with tc.tile_pool(name="w2p", bufs=2) as w2p, \
             tc.tile_pool(name="hhp", bufs=3) as hhp, \
             tc.tile_pool(name="ps_y2", bufs=1, space="PSUM") as ps_y2, \
             tc.tile_pool(name="osb", bufs=2) as osb:
            per_pass = max(1, 8 // TB)  # col tiles per pass (8 psum banks)
            for p0 in range(0, CT if SA >= 8 else 0, per_pass):
                cts = list(range(p0, min(CT, p0 + per_pass)))
                pw = len(cts) * CW
                ps_t = {(tb, ct): ps_y2.tile([128, CW], F32,
                                             name=f"y2_{tb}_{ct - p0}")
                        for tb in range(TB) for ct in cts}
                for gf in range(FFB // W2G):
                    wt = w2p.tile([128, W2G, pw], BF, name="w2_t")
                    nc.sync.dma_start(
                        wt[:], w2_io[gf, :, :, p0 * CW:p0 * CW + pw])
                    for fi in range(W2G):
                        fc = gf * W2G + fi
                        if cfg.hh_sbuf:
                            def hh_s(tb, fc=fc):
                                return hh_sb[:, fc,
                                             tb * 128:(tb + 1) * 128]
                        else:
                            hht = hhp.tile([128, T], BF, name="hh_s")
                            nc.sync.dma_start(
                                hht[:], hh_d[fc * 128:(fc + 1) * 128, :])

                            def hh_s(tb, hht=hht):
                                return hht[:, tb * 128:(tb + 1) * 128]
                        for tb in range(TB):
                            for ct in cts:
                                o = (ct - p0) * CW
                                nc.tensor.matmul(
                                    ps_t[(tb, ct)][:],
                                    hh_s(tb),
                                    wt[:, fi, o:o + CW],
                                    start=(fc == 0), stop=(fc == FFB - 1))
                for tb in range(TB):
                    for ct in cts:
                        c0 = ct * CW
                        o = osb.tile([128, CW], F32, name="o_sb")
                        nc.vector.tensor_add(o[:], ps_t[(tb, ct)][:],
                                             x1_t[tb][:, c0:c0 + CW])
                        nc.sync.dma_start(
                            out_io[tb * 128:(tb + 1) * 128, c0:c0 + CW], o[:])

        if SA < 8:  # truncated probe build: emit dummy output
            for tb in range(TB):
                nc.any.memset(x1_t[tb][:, 0:1], 0.0)
                nc.sync.dma_start(out_io[tb * 128:(tb + 1) * 128, :],
                                  x1_t[tb][:])

    nc.compile()
    return nc


# --------------------------- host-side prep ---------------------------

def host_prep(cfg: Cfg, x, w_qkv, w_out, w1, w2, w3, g1, g2):
    """Build the per-core input maps (numpy, bf16 weights, mask data)."""
    n, D, H, DFF = cfg.n_cores, cfg.D, cfg.H, cfg.DFF
    S_blk, DC, NK, QB, FFB = cfg.S_blk, cfg.DC, cfg.NK, cfg.QB, cfg.FFB
    GW, CW, CT, W2G, T = cfg.GW, cfg.CW, cfg.CT, cfg.W2G, cfg.T

    def group_layout(w, gw):
        # [D, C] -> [C//gw, 128, DC, gw]
        C = w.shape[1]
        return np.ascontiguousarray(
            w.reshape(DC, 128, C // gw, gw).transpose(2, 1, 0, 3))

    x = np.asarray(x, np.float32)
    g1 = np.asarray(g1, np.float32)
    g2 = np.asarray(g2, np.float32)

    # softmax scale Hd^-0.5 is applied in the kernel's exp (not folded here)
    wqkv = np.asarray(w_qkv, np.float32) * g1[:, None]
    if cfg.fp8_qkv:
        wqkv_t = np.clip(group_layout(wqkv, GW) * S_W,
                         -240.0, 240.0).astype(E4M3)
    else:
        wqkv_t = group_layout(wqkv, GW).astype(BF16)

    wout = np.asarray(w_out, np.float32)
    if cfg.fp8_out:
        wout_t = np.clip(group_layout(wout, CW) * S_W,
                         -240.0, 240.0).astype(E4M3)
    else:
        wout_t = group_layout(wout, CW).astype(BF16)

    w1g = (np.asarray(w1, np.float32) * g2[:, None]).reshape(DC, 128, FFB, 128)
    w3g = (np.asarray(w3, np.float32) * g2[:, None]).reshape(DC, 128, FFB, 128)
    w13 = np.stack([w1g, w3g], axis=3).reshape(DC, 128, 2 * DFF)
    w13_t = group_layout(w13.reshape(DC * 128, 2 * DFF), 512).astype(BF16)

    w2_t = np.ascontiguousarray(
        np.asarray(w2, np.float32).reshape(FFB // W2G, W2G, 128, D)
        .transpose(0, 2, 1, 3)).astype(BF16)

    ident = np.eye(128, dtype=np.float32).astype(BF16)
    ones128 = np.ones((128, 128), np.float32).astype(BF16)

    # local order: l = blk*256 + batch*128 + i ; token = n*(blk*128+i) + c
    l = np.arange(cfg.T)
    blk, qb_a, i_a = l // 256, (l // 128) % 2, l % 128
    base_pos = n * (blk * 128 + i_a)

    in_maps = []
    for c in range(n):
        pos = base_pos + c
        x_c = np.ascontiguousarray(x[qb_a, pos, :])
        # diagonal-chunk masks: key (p, rank r) vs query (q, core c):
        # allowed iff p < q or (p == q and r <= c)
        masks = np.zeros((128, n // 2, 2, 256), np.float32)
        kp = np.arange(128)[:, None]
        qq = np.arange(128)[None, :]
        for r in range(n):
            m = (kp < qq) | ((kp == qq) & (r <= c))
            masks[:, r // 2, r % 2, 0:128] = m
            masks[:, r // 2, r % 2, 128:256] = m
        in_maps.append({
            "x_c": x_c,
            "wqkv_t": wqkv_t, "wout_t": wout_t, "w13_t": w13_t, "w2_t": w2_t,
            "masks": masks.astype(E4M3 if cfg.fp8_att else BF16),
            "ident": ident, "ones128": ones128,
        })
    return in_maps


def assemble(cfg: Cfg, results):
    n = cfg.n_cores
    out = np.empty((cfg.B, cfg.S, cfg.D), np.float32)
    l = np.arange(cfg.T)
    blk, qb_a, i_a = l // 256, (l // 128) % 2, l % 128
    base_pos = n * (blk * 128 + i_a)
    for c in range(n):
        out[qb_a, base_pos + c] = results[c]["out_c"]
    return out


_NC_CACHE = {}


def get_nc(cfg: Cfg = FULL):
    if cfg not in _NC_CACHE:
        _NC_CACHE[cfg] = build_nc(cfg)
    return _NC_CACHE[cfg]


def kernel(x, w_qkv, w_out, w1, w2, w3, g1, g2):
    cfg = FULL
    nc = get_nc(cfg)
    in_maps = host_prep(cfg, x, w_qkv, w_out, w1, w2, w3, g1, g2)
    res = run_bass_kernel_spmd(nc, in_maps, core_ids=list(range(cfg.n_cores)))
    return assemble(cfg, res.results)



# revision 75
# speedup vs baseline: 1.3290x; 1.1252x over previous
"""Trainium2 Bass kernel for a dense transformer block (RMSNorm -> causal MHA
-> residual -> RMSNorm -> SwiGLU FFN -> residual).

Sharding: data-parallel over tokens with a *strided* assignment -- core c
owns every token position == c (mod 8) of both batches (512 tokens/core),
weights replicated. The stride makes the causal chunk structure identical on
every core (no padding waste), so one SPMD program serves all cores; the
residual per-core causality lands in small per-core mask *data* for the
diagonal key chunks only. K/V are exchanged with one AllGather pair.

Matmuls run in bf16 with fp32 PSUM accumulation; residuals/norms stay fp32.
Attention processes chunk pairs for both batches at once ([128, 512] exp and
mask tiles), softmax denominators accumulate on the PE via a ones-column
matmul, and 1/r normalization is applied via a PE partition-broadcast.
"""

from dataclasses import dataclass
from contextlib import ExitStack

import numpy as np

import concourse.bacc as bacc
import concourse.mybir as mybir
import concourse.tile as tile
from concourse.bass_utils import run_bass_kernel_spmd

try:
    import ml_dtypes

    BF16 = ml_dtypes.bfloat16
    E4M3 = ml_dtypes.float8_e4m3  # TRN fp8e4: max +-240, matches <=240
except ImportError:  # pragma: no cover
    import jax.numpy as jnp

    BF16 = jnp.bfloat16
    E4M3 = jnp.float8_e4m3

F32 = mybir.dt.float32
BF = mybir.dt.bfloat16
FP8 = mybir.dt.float8e4
AF = mybir.ActivationFunctionType
DR = mybir.MatmulPerfMode.DoubleRow

# fp8 scaling: weights/activations are pre-scaled out of the subnormal range
# (w sigma ~ 1/sqrt(D), xn ~ unit RMS); descale on PSUM copy-out.
S_W = 128.0  # qkv / out weight scale
S_X = 16.0   # normalized-activation scale
S_O = 16.0   # attention-output scale
S_V = 16.0   # V scale for fp8 attention weights
S_QK = 16.0  # Q/K scale for fp8 scores
C_SHIFT = 3.0  # exp shift: e' = exp(s/sqrt(Hd) - C); cancels in softmax


@dataclass(frozen=True)
class Cfg:
    B: int = 2
    S: int = 2048
    D: int = 2048
    H: int = 16
    DFF: int = 8192
    n_cores: int = 8
    eps: float = 1e-6
    use_silu: bool = False  # ACT Silu table vs sigmoid+mul (sim lacks Silu)
    fake_ag: bool = False   # replace AllGather with local DMA (TimelineSim)
    fp8_qkv: bool = True    # qkv projection in fp8e4 DoubleRow (2x PE)
    fp8_out: bool = True    # attention out-projection in fp8e4 DoubleRow
    pool_rsum: bool = False  # softmax denominators: accumulate e on Pool
    fp8_att: bool = True    # shifted-exp fp8 attn weights + fp8 V (DR av/rsum)
    fp8_qk: bool = True     # fp8 Q/K (fp8 scores; K travels fp8 on the wire)
    merged_ag: bool = False  # single AllGather carrying K and V (slower:
    #   2MB/rank leaves the <1MB low-latency mesh collective regime)
    hh_sbuf: bool = True    # keep FFN hidden activations SBUF-resident
    stop_after: int = 0     # timing probe: truncate after phase N (0 = full)

    @property
    def Hd(self):
        return self.D // self.H

    @property
    def S_blk(self):
        return self.S // self.n_cores  # per-(core,batch) query block

    @property
    def T(self):
        return self.B * self.S_blk  # tokens per core

    @property
    def TB(self):
        return self.T // 128

    @property
    def DC(self):
        return self.D // 128

    @property
    def NK(self):
        return self.S // 128  # padded key chunks per batch

    @property
    def QB(self):
        return self.S_blk

    @property
    def FFB(self):
        return self.DFF // 128

    @property
    def CT(self):
        return max(1, self.D // 512)

    @property
    def CW(self):
        return min(512, self.D)

    @property
    def GW(self):
        return min(512, self.D)  # qkv weight col group width

    @property
    def W2G(self):
        return 4  # w2 chunks per DMA group


FULL = Cfg()


def build_nc(cfg: Cfg):
    """Build the per-core Bass program (identical on all cores)."""
    assert cfg.Hd == 128 and cfg.S_blk % 128 == 0 and cfg.T % 128 == 0
    n, D, H, T, DC, NK, QB, FFB = (
        cfg.n_cores, cfg.D, cfg.H, cfg.T, cfg.DC, cfg.NK, cfg.QB, cfg.FFB)
    S_blk, TB, CT, CW, DFF, GW, W2G = (
        cfg.S_blk, cfg.TB, cfg.CT, cfg.CW, cfg.DFF, cfg.GW, cfg.W2G)
    SB2 = S_blk // 128
    assert FFB % W2G == 0 and (3 * D) % GW == 0
    assert n % 4 == 0 and T % 256 == 0

    nc = bacc.Bacc("TRN2", target_bir_lowering=False, debug=False,
                   num_devices=n)

    # ---- I/O (host pre-tiled layouts; one DMA per weight group) ----
    x_io = nc.dram_tensor("x_c", [T, D], F32, kind="ExternalInput")
    wqkv_io = nc.dram_tensor("wqkv_t", [3 * D // GW, 128, DC, GW],
                             FP8 if cfg.fp8_qkv else BF,
                             kind="ExternalInput")
    wout_io = nc.dram_tensor("wout_t", [CT, 128, DC, CW],
                             FP8 if cfg.fp8_out else BF,
                             kind="ExternalInput")
    w13_io = nc.dram_tensor("w13_t", [FFB // 2, 128, DC, 512], BF,
                            kind="ExternalInput")
    w2_io = nc.dram_tensor("w2_t", [FFB // W2G, 128, W2G, D], BF,
                           kind="ExternalInput")
    masks_io = nc.dram_tensor("masks", [128, n // 2, 2, 256],
                              FP8 if cfg.fp8_att else BF,
                              kind="ExternalInput")
    ident_io = nc.dram_tensor("ident", [128, 128], BF, kind="ExternalInput")
    ones_io = nc.dram_tensor("ones128", [128, 128], BF, kind="ExternalInput")
    out_io = nc.dram_tensor("out_c", [T, D], F32, kind="ExternalOutput")

    SA = cfg.stop_after if cfg.stop_after else 99  # phase truncation probe
    with tile.TileContext(nc) as tc, ExitStack() as top:
        P = top.enter_context(tc.tile_pool(name="persist", bufs=1))
        consts = top.enter_context(tc.tile_pool(name="consts", bufs=1))

        ident = consts.tile([128, 128], BF, name="ident_sb")
        ones = consts.tile([128, 128], BF, name="ones_sb")
        mask_sb = consts.tile([128, n // 2, 2, 256],
                              FP8 if cfg.fp8_att else BF, name="mask_sb")
        eps_t = consts.tile([128, 1], F32, name="eps_sb")
        nc.sync.dma_start(ident[:], ident_io[:, :])
        nc.any.memset(eps_t[:], cfg.eps)
        if cfg.fp8_att:
            # all-ones fp8 weights, 128 out-columns: the denominator matmul
            # rp = ones8^T @ e lands broadcast across all 128 partitions
            ones8 = consts.tile([128, 2, 128], FP8, name="ones8_sb")
            nc.any.memset(ones8[:], 1.0)
            negc = consts.tile([128, 1], F32, name="negc_sb")
            nc.any.memset(negc[:], -C_SHIFT)

        x1_t = [P.tile([128, D], F32, name=f"x1_{tb}") for tb in range(TB)]
        xzt = P.tile([128, DC, T], BF, name="xzt")       # z2^T for FFN
        xzt8 = (P.tile([128, DC, T], FP8, name="xzt8")   # xn^T fp8 for QKV
                if cfg.fp8_qkv else xzt)

        # attention-scoped persistents: freed before the FFN phases so the
        # SwiGLU hidden state can live in SBUF
        attn_stack = ExitStack()
        AP = attn_stack.enter_context(tc.tile_pool(name="attnP", bufs=1))
        QKDT = FP8 if cfg.fp8_qk else BF
        qt = AP.tile([128, H, T], QKDT, name="qt")       # Q d-major
        v_all = AP.tile([128, TB, D], FP8 if cfg.fp8_att else BF,
                        name="v_all")                    # V token-major
        ot = AP.tile([128, H, T], FP8 if cfg.fp8_out else BF,
                     name="ot")                          # attn out d-major

        dram = top.enter_context(tc.tile_pool(name="dram", bufs=1, space="DRAM"))
        VDT = FP8 if cfg.fp8_att else BF
        if cfg.merged_ag:
            assert cfg.fp8_qk and cfg.fp8_att
            # fp8 [4096, 512]: rows [0, D) = K^T; rows [D, 2D) = V flattened
            # (token t = 4 rows of 512 features each)
            kv_c = dram.tile([2 * D, T], FP8, name="kv_contrib")
            kv_g = dram.tile([n * 2 * D, T], FP8, name="kv_gath",
                             addr_space="Shared")
        else:
            kt_c = dram.tile([D, T], QKDT, name="kt_contrib")
            v_c = dram.tile([T, D], VDT, name="v_contrib")
            kt_g = dram.tile([n * D, T], QKDT, name="kt_gath",
                             addr_space="Shared")
            v_g = dram.tile([n * T, D], VDT, name="v_gath",
                            addr_space="Shared")
        if not cfg.hh_sbuf:
            hh_d = dram.tile([DFF, T], BF, name="hh_d")  # swiglu spill

        def rmsnorm_transpose(get_src, ps_tp, pool, dst, out_scale=1.0):
            """token-major fp32 tiles -> d-major bf16/fp8 into dst."""
            for tb in range(TB):
                xt = get_src(tb)
                scr = pool.tile([128, D], BF, name="nrm_scr")
                ssq = pool.tile([128, 1], F32, name="nrm_ssq")
                nc.scalar.activation(scr[:], xt[:], AF.Square,
                                     accum_out=ssq[:])
                sd = pool.tile([128, 1], F32, name="nrm_sd")
                nc.scalar.activation(sd[:], ssq[:], AF.Sqrt,
                                     bias=eps_t[:], scale=1.0 / D)
                inv = pool.tile([128, 1], F32, name="nrm_inv")
                nc.vector.reciprocal(inv[:], sd[:])
                xn = pool.tile([128, D], BF, name="nrm_xn")
                nc.vector.tensor_scalar_mul(xn[:], xt[:], inv[:])
                for dc in range(DC):
                    tp = ps_tp.tile([128, 128], BF, name="tp")
                    nc.tensor.transpose(tp[:], xn[:, dc * 128:(dc + 1) * 128],
                                        ident[:])
                    d = dst[:, dc, tb * 128:(tb + 1) * 128]
                    if out_scale != 1.0:
                        nc.scalar.mul(d, tp[:], out_scale)
                    else:
                        nc.vector.tensor_copy(d, tp[:])

        # ================= phase 1: rmsnorm1 + transpose =================
        with tc.tile_pool(name="ph1", bufs=2) as ph1, \
             tc.tile_pool(name="ps_tp1", bufs=4, space="PSUM") as ps_tp1:

            def load_x(tb):
                t = ph1.tile([128, D], F32, name=f"xld_{tb}")
                nc.sync.dma_start(t[:], x_io[tb * 128:(tb + 1) * 128, :])
                return t

            rmsnorm_transpose(load_x, ps_tp1, ph1, xzt8,
                              S_X if cfg.fp8_qkv else 1.0)
            # consts not needed until attention -- keep them off the DMA
            # queue ahead of the x loads
            nc.sync.dma_start(ones[:], ones_io[:, :])
            nc.sync.dma_start(mask_sb[:], masks_io[:, :, :, :])

        # ============== phase 2: qkv projections + AllGather ==============
        # order: K, V (feed the AllGather), then Q (overlaps the AG)
        with tc.tile_pool(name="ph2", bufs=1) as ph2, \
             tc.tile_pool(name="wg", bufs=3) as wg, \
             tc.tile_pool(name="ps_mm", bufs=3, space="PSUM") as ps_mm:
            kt_l = ph2.tile([128, H, T], QKDT, name="kt_l")  # local K d-major
            qkscale = ((S_QK if cfg.fp8_qk else 1.0)
                       / ((S_W * S_X) if cfg.fp8_qkv else 1.0))

            def load_group(g):
                t = wg.tile([128, DC, GW], FP8 if cfg.fp8_qkv else BF,
                            name="wg_t")
                nc.sync.dma_start(t[:], wqkv_io[g, :, :, :])
                return t

            def qk_block(wt, off, dst, slot):
                ps = ps_mm.tile([128, T], F32, name="ps_qk")
                if cfg.fp8_qkv:
                    for dc in range(0, DC, 2):
                        nc.tensor.matmul(ps[:], wt[:, dc:dc + 2, off:off + 128],
                                         xzt8[:, dc:dc + 2, :],
                                         start=(dc == 0), stop=(dc == DC - 2),
                                         perf_mode=DR)
                else:
                    for dc in range(DC):
                        nc.tensor.matmul(ps[:], wt[:, dc, off:off + 128],
                                         xzt[:, dc, :],
                                         start=(dc == 0), stop=(dc == DC - 1))
                nc.scalar.mul(dst[:, slot, :], ps[:], qkscale)

            # K: wqkv cols [D, 2D)
            for g in range(D // GW, 2 * D // GW if SA >= 2 else 0):
                wt = load_group(g)
                for b in range(GW // 128):
                    h = g * GW // 128 + b - H
                    qk_block(wt, b * 128, kt_l, h)
            # V: wqkv cols [2D, 3D) -> token-major
            for g in range(2 * D // GW, 3 * D // GW if SA >= 2 else 0):
                wt = load_group(g)
                base = g * GW - 2 * D
                vscale = ((S_V if cfg.fp8_att else 1.0)
                          / ((S_W * S_X) if cfg.fp8_qkv else 1.0))
                for tb in range(TB):
                    for c0 in range(0, GW, CW):
                        ps = ps_mm.tile([128, CW], F32, name="ps_v")
                        if cfg.fp8_qkv:
                            for dc in range(0, DC, 2):
                                nc.tensor.matmul(
                                    ps[:],
                                    xzt8[:, dc:dc + 2, tb * 128:(tb + 1) * 128],
                                    wt[:, dc:dc + 2, c0:c0 + CW],
                                    start=(dc == 0), stop=(dc == DC - 2),
                                    perf_mode=DR)
                        else:
                            for dc in range(DC):
                                nc.tensor.matmul(
                                    ps[:], xzt[:, dc, tb * 128:(tb + 1) * 128],
                                    wt[:, dc, c0:c0 + CW],
                                    start=(dc == 0), stop=(dc == DC - 1))
                        nc.scalar.mul(
                            v_all[:, tb, base + c0:base + c0 + CW], ps[:],
                            vscale)

            for h in range(H if SA >= 2 else 0):
                dst = kv_c if cfg.merged_ag else kt_c
                nc.sync.dma_start(dst[h * 128:(h + 1) * 128, :],
                                  kt_l[:, h, :])
            for tb in range(TB if SA >= 2 else 0):
                if cfg.merged_ag:
                    nc.sync.dma_start(
                        kv_c[D + tb * 512:D + (tb + 1) * 512, :]
                        .rearrange("(p a) q -> p a q", a=4),
                        v_all[:, tb, :]
                        .rearrange("p (a q) -> p a q", a=4))
                else:
                    nc.sync.dma_start(v_c[tb * 128:(tb + 1) * 128, :],
                                      v_all[:, tb, :])
            if SA < 2:
                pass
            elif cfg.fake_ag:
                if cfg.merged_ag:
                    nc.sync.dma_start(kv_g[0:2 * D, :], kv_c[:, :])
                else:
                    nc.sync.dma_start(kt_g[0:D, :], kt_c[:, :])
                    nc.sync.dma_start(v_g[0:T, :], v_c[:, :])
            elif cfg.merged_ag:
                nc.gpsimd.collective_compute(
                    "AllGather", mybir.AluOpType.bypass,
                    replica_groups=[list(range(n))],
                    ins=[kv_c.opt()], outs=[kv_g.opt()])
            else:
                nc.gpsimd.collective_compute(
                    "AllGather", mybir.AluOpType.bypass,
                    replica_groups=[list(range(n))],
                    ins=[kt_c.opt()], outs=[kt_g.opt()])
                nc.gpsimd.collective_compute(
                    "AllGather", mybir.AluOpType.bypass,
                    replica_groups=[list(range(n))],
                    ins=[v_c.opt()], outs=[v_g.opt()])

            # Q: wqkv cols [0, D)  (overlaps the AG)
            for g in range(0, D // GW if SA >= 2 else 0):
                wt = load_group(g)
                for b in range(GW // 128):
                    h = g * GW // 128 + b
                    qk_block(wt, b * 128, qt, h)

        # ================= phase 4: attention =================
        # Strided token assignment: core c owns tokens == c (mod n); local
        # order is l = blk*256 + batch*128 + i  (token = n*(blk*128+i) + c).
        # Every local 256-query super-block b needs exactly (b+1)*n key
        # chunks on every core -- no padding. Only diagonal (j2 == b) chunks
        # need a mask; mask content is per-core data. Chunks are processed
        # in pairs so exp/mask run on [128, 512] tiles.
        HG = min(4, H)
        NP2 = n // 2
        SPG = 2 if cfg.fp8_att else 1  # score pairs fused per exp call
        with tc.tile_pool(name="kv", bufs=2) as kv, \
             tc.tile_pool(name="ktp", bufs=3) as ktp, \
             tc.tile_pool(name="esb", bufs=6 // SPG) as esb, \
             tc.tile_pool(name="esump", bufs=2) as esump, \
             tc.tile_pool(name="aux", bufs=2) as aux, \
             tc.tile_pool(name="ps_s", bufs=4 // SPG, space="PSUM") as ps_s, \
             tc.tile_pool(name="ps_ot", bufs=1, space="PSUM") as ps_ot, \
             tc.tile_pool(name="ps_r", bufs=1, space="PSUM") as ps_r, \
             tc.tile_pool(name="ps_b", bufs=1, space="PSUM") as ps_b:
            for hq in range(H // HG if SA >= 4 else 0):
                vtb = [kv.tile([128, NK, HG * 128],
                               FP8 if cfg.fp8_att else BF, name=f"vtb{qb}")
                       for qb in range(2)]
                for qb in range(2):
                    for r in range(n):
                        for j2 in range(SB2):
                            t0 = j2 * 256 + qb * 128
                            if cfg.merged_ag:
                                base = r * 2 * D + D
                                src = (kv_g[base + 4 * t0:
                                            base + 4 * (t0 + 128), :]
                                       .rearrange("(p a) q -> p a q", a=4)
                                       [:, hq, :])
                            else:
                                src = v_g[r * T + t0:r * T + t0 + 128,
                                          hq * HG * 128:(hq + 1) * HG * 128]
                            nc.sync.dma_start(vtb[qb][:, j2 * n + r, :], src)
                for hi in range(HG):
                    h = hq * HG + hi
                    ktb = ktp.tile([128, n, T], QKDT, name="ktb")
                    for r in range(n):
                        if cfg.merged_ag:
                            src = kv_g[r * 2 * D + h * 128:
                                       r * 2 * D + (h + 1) * 128, :]
                        else:
                            src = kt_g[r * D + h * 128:r * D + (h + 1) * 128,
                                       :]
                        nc.sync.dma_start(ktb[:, r, :], src)
                    for b in range(SB2):
                        otp = [ps_ot.tile([128, 128], F32, name=f"otp{qb}")
                               for qb in range(2)]
                        rp = ps_r.tile(
                            [128, 256] if cfg.fp8_att else [1, 512], F32,
                            name="rp")
                        npairs = (b + 1) * NP2
                        pi = 0
                        for j2 in range(b + 1 if cfg.fp8_att else 0):
                            # fp8 path: 2 pairs (4 key chunks) per exp/mask
                            for pg in range(NP2 // SPG):
                                sp = ps_s.tile([128, SPG, 2, 256], F32,
                                               name="sp")
                                for c in range(2 * SPG):
                                    r = 2 * SPG * pg + c
                                    for qb in range(2):
                                        nc.tensor.matmul(
                                            sp[:, c // 2, c % 2,
                                               qb * 128:(qb + 1) * 128],
                                            ktb[:, r,
                                                j2 * 256 + qb * 128:
                                                j2 * 256 + (qb + 1) * 128],
                                            qt[:, h,
                                               b * 256 + qb * 128:
                                               b * 256 + (qb + 1) * 128],
                                            start=True, stop=True)
                                e = esb.tile([128, SPG, 2, 256], FP8,
                                             name="e")
                                nc.scalar.activation(
                                    e[:], sp[:], AF.Exp,
                                    scale=cfg.Hd ** -0.5
                                    / (S_QK * S_QK if cfg.fp8_qk else 1.0),
                                    bias=negc[:])
                                if j2 == b:
                                    nc.vector.tensor_mul(
                                        e[:], e[:],
                                        mask_sb[:, SPG * pg:SPG * (pg + 1),
                                                :, :])
                                for g2 in range(SPG):
                                    nc.tensor.matmul(
                                        rp[:], ones8[:, :, :],
                                        e[:, g2, :, :],
                                        start=(pi == 0),
                                        stop=(pi == npairs - 1),
                                        perf_mode=DR)
                                    for qb in range(2):
                                        ck = j2 * n + 2 * SPG * pg + 2 * g2
                                        nc.tensor.matmul(
                                            otp[qb][:],
                                            vtb[qb][:, ck:ck + 2,
                                                    hi * 128:(hi + 1) * 128],
                                            e[:, g2, :,
                                              qb * 128:(qb + 1) * 128],
                                            start=(pi == 0),
                                            stop=(pi == npairs - 1),
                                            perf_mode=DR)
                                    pi += 1
                        for j2 in range(0 if cfg.fp8_att else b + 1):
                            for pm in range(NP2):
                                sp = ps_s.tile([128, 2, 256], F32, name="sp")
                                for ci in range(2):
                                    r = 2 * pm + ci
                                    for qb in range(2):
                                        nc.tensor.matmul(
                                            sp[:, ci,
                                               qb * 128:(qb + 1) * 128],
                                            ktb[:, r,
                                                j2 * 256 + qb * 128:
                                                j2 * 256 + (qb + 1) * 128],
                                            qt[:, h,
                                               b * 256 + qb * 128:
                                               b * 256 + (qb + 1) * 128],
                                            start=True, stop=True)
                                e = esb.tile([128, 2, 256], BF, name="e")
                                nc.scalar.activation(
                                    e[:], sp[:], AF.Exp,
                                    scale=cfg.Hd ** -0.5
                                    / (S_QK * S_QK if cfg.fp8_qk else 1.0))
                                if j2 == b:
                                    nc.vector.tensor_mul(
                                        e[:], e[:], mask_sb[:, pm, :, :])
                                if cfg.pool_rsum:
                                    # denominators: accumulate e on the (idle)
                                    # Pool engine; one ones-matmul at the end
                                    if pi == 0:
                                        esum = esump.tile([128, 2, 256], BF,
                                                          name="esum")
                                        nc.gpsimd.tensor_copy(esum[:], e[:])
                                    else:
                                        nc.gpsimd.tensor_add(esum[:],
                                                             esum[:], e[:])
                                else:
                                    nc.tensor.matmul(
                                        rp[:], ones[:, 0:1], e[:],
                                        start=(pi == 0),
                                        stop=(pi == npairs - 1))
                                for ci in range(2):
                                    r = 2 * pm + ci
                                    for qb in range(2):
                                        nc.tensor.matmul(
                                            otp[qb][:],
                                            vtb[qb][:, j2 * n + r,
                                                    hi * 128:(hi + 1) * 128],
                                            e[:, ci, qb * 128:(qb + 1) * 128],
                                            start=(pi == 0 and ci == 0),
                                            stop=(pi == npairs - 1
                                                  and ci == 1))
                                pi += 1
                        if cfg.pool_rsum and not cfg.fp8_att:
                            nc.tensor.matmul(rp[:], ones[:, 0:1], esum[:],
                                             start=True, stop=True)
                        if cfg.fp8_att:
                            # rp already holds r broadcast across partitions
                            rinv = aux.tile([128, 256], F32, name="rinv128")
                            nc.vector.reciprocal(rinv[:], rp[:])
                            if S_O != S_V:
                                nc.scalar.mul(rinv[:], rinv[:], S_O / S_V)
                            for qb in range(2):
                                nc.vector.tensor_mul(
                                    ot[:, h, b * 256 + qb * 128:
                                       b * 256 + (qb + 1) * 128],
                                    otp[qb][:],
                                    rinv[:, qb * 128:(qb + 1) * 128])
                            continue
                        rsum = aux.tile([1, 256], F32, name="rsum")
                        nc.vector.tensor_reduce(
                            rsum[:],
                            rp[:].rearrange("p (a q) -> p q a", a=2),
                            axis=mybir.AxisListType.X,
                            op=mybir.AluOpType.add)
                        rinv = aux.tile([1, 256], F32, name="rinv")
                        nc.vector.reciprocal(rinv[:], rsum[:])
                        rinv_b = aux.tile([1, 256], BF, name="rinv_b")
                        nc.scalar.mul(rinv_b[:], rinv[:],
                                      S_O if cfg.fp8_out else 1.0)
                        rbc = ps_b.tile([128, 256], F32, name="rbc")
                        nc.tensor.matmul(rbc[:], ones[0:1, :], rinv_b[:],
                                         start=True, stop=True)
                        rbc_sb = aux.tile([128, 256], BF, name="rbc_sb")
                        nc.scalar.copy(rbc_sb[:], rbc[:])
                        for qb in range(2):
                            nc.vector.tensor_mul(
                                ot[:, h,
                                   b * 256 + qb * 128:b * 256 + (qb + 1) * 128],
                                otp[qb][:], rbc_sb[:, qb * 128:(qb + 1) * 128])

        # ============== phase 5: out-proj + residual ==============
        # tb outer so each token block's x1 completes early for rmsnorm2
        with tc.tile_pool(name="ph5", bufs=2) as ph5, \
             tc.tile_pool(name="wo5", bufs=1) as wo5, \
             tc.tile_pool(name="ps_y", bufs=2, space="PSUM") as ps_y:
            wo_gs = []
            for ct in range(CT if SA >= 5 else 0):
                wo_g = wo5.tile([128, DC, CW], FP8 if cfg.fp8_out else BF,
                                name=f"wo_g{ct}")
                nc.sync.dma_start(wo_g[:], wout_io[ct, :, :, :])
                wo_gs.append(wo_g)
            for tb in range(TB if SA >= 5 else 0):
                for ct in range(CT):
                    c0 = ct * CW
                    wo_g = wo_gs[ct]
                    ps = ps_y.tile([128, CW], F32, name="ps_y")
                    if cfg.fp8_out:
                        for hc in range(0, H, 2):
                            nc.tensor.matmul(
                                ps[:],
                                ot[:, hc:hc + 2, tb * 128:(tb + 1) * 128],
                                wo_g[:, hc:hc + 2, :],
                                start=(hc == 0), stop=(hc == H - 2),
                                perf_mode=DR)
                        yv = ph5.tile([128, CW], BF, name="yv")
                        nc.scalar.mul(yv[:], ps[:], 1.0 / (S_W * S_O))
                        src = yv
                    else:
                        for hc in range(H):
                            nc.tensor.matmul(
                                ps[:], ot[:, hc, tb * 128:(tb + 1) * 128],
                                wo_g[:, hc, :],
                                start=(hc == 0), stop=(hc == H - 1))
                        src = ps
                    xr = ph5.tile([128, CW], F32, name="xr")
                    nc.sync.dma_start(
                        xr[:], x_io[tb * 128:(tb + 1) * 128, c0:c0 + CW])
                    nc.vector.tensor_add(x1_t[tb][:, c0:c0 + CW], src[:],
                                         xr[:])

        attn_stack.close()  # free qt / v_all / ot for the FFN hidden state
        if cfg.hh_sbuf:
            hhP = top.enter_context(tc.tile_pool(name="hhP", bufs=1))
            hh_sb = hhP.tile([128, FFB, T], BF, name="hh_sb")

        # ============== phase 6: rmsnorm2 + transpose ==============
        with tc.tile_pool(name="ph6", bufs=2) as ph6, \
             tc.tile_pool(name="ps_tp6", bufs=4, space="PSUM") as ps_tp6:
            if SA >= 6:
                rmsnorm_transpose(lambda tb: x1_t[tb], ps_tp6, ph6, xzt)

        # ============== phase 7: FFN up (w1/w3 + swiglu) ==============
        with tc.tile_pool(name="ph7", bufs=3) as ph7, \
             tc.tile_pool(name="w13", bufs=3) as w13p, \
             tc.tile_pool(name="ps_h", bufs=3, space="PSUM") as ps_h:
            for g in range(FFB // 2 if SA >= 7 else 0):  # 512-col groups
                wt = w13p.tile([128, DC, 512], BF, name="w13_t")
                nc.sync.dma_start(wt[:], w13_io[g, :, :, :])
                for fi in range(2):
                    f = 2 * g + fi
                    o1, o3 = fi * 256, fi * 256 + 128
                    h1 = ps_h.tile([128, T], F32, name="h1")
                    for dc in range(DC):
                        nc.tensor.matmul(h1[:], wt[:, dc, o1:o1 + 128],
                                         xzt[:, dc, :],
                                         start=(dc == 0), stop=(dc == DC - 1))
                    s1 = ph7.tile([128, T], BF, name="s1")
                    if cfg.use_silu:
                        nc.scalar.activation(s1[:], h1[:], AF.Silu)
                    else:
                        sg = ph7.tile([128, T], BF, name="sg")
                        nc.scalar.activation(sg[:], h1[:], AF.Sigmoid)
                        nc.vector.tensor_mul(s1[:], sg[:], h1[:])
                    h3 = ps_h.tile([128, T], F32, name="h3")
                    for dc in range(DC):
                        nc.tensor.matmul(h3[:], wt[:, dc, o3:o3 + 128],
                                         xzt[:, dc, :],
                                         start=(dc == 0), stop=(dc == DC - 1))
                    if cfg.hh_sbuf:
                        nc.vector.tensor_mul(hh_sb[:, f, :], s1[:], h3[:])
                    else:
                        hh = ph7.tile([128, T], BF, name="hh")
                        nc.vector.tensor_mul(hh[:], s1[:], h3[:])
                        nc.sync.dma_start(hh_d[f * 128:(f + 1) * 128, :],
                                          hh[:])

        # ============== phase 8: FFN down + residual + out ==============
        with tc.tile_pool(name="w2p", bufs=3) as w2p, \
             tc.tile_pool(name="hhp", bufs=3) as hhp, \
             tc.tile_pool(name="ps_y2", bufs=1, space="PSUM") as ps_y2, \
             tc.tile_pool(name="osb", bufs=2) as osb:
            per_pass = max(1, 8 // TB)  # col tiles per pass (8 psum banks)
            for p0 in range(0, CT if SA >= 8 else 0, per_pass):
                cts = list(range(p0, min(CT, p0 + per_pass)))
                pw = len(cts) * CW
                ps_t = {(tb, ct): ps_y2.tile([128, CW], F32,
                                             name=f"y2_{tb}_{ct - p0}")
                        for tb in range(TB) for ct in cts}
                for gf in range(FFB // W2G):
                    wt = w2p.tile([128, W2G, pw], BF, name="w2_t")
                    nc.sync.dma_start(
                        wt[:], w2_io[gf, :, :, p0 * CW:p0 * CW + pw])
                    for fi in range(W2G):
                        fc = gf * W2G + fi
                        if cfg.hh_sbuf:
                            def hh_s(tb, fc=fc):
                                return hh_sb[:, fc,
                                             tb * 128:(tb + 1) * 128]
                        else:
                            hht = hhp.tile([128, T], BF, name="hh_s")
                            nc.sync.dma_start(
                                hht[:], hh_d[fc * 128:(fc + 1) * 128, :])

                            def hh_s(tb, hht=hht):
                                return hht[:, tb * 128:(tb + 1) * 128]
                        for tb in range(TB):
                            for ct in cts:
                                o = (ct - p0) * CW
                                nc.tensor.matmul(
                                    ps_t[(tb, ct)][:],
                                    hh_s(tb),
                                    wt[:, fi, o:o + CW],
                                    start=(fc == 0), stop=(fc == FFB - 1))
                for tb in range(TB):
                    for ct in cts:
                        c0 = ct * CW
                        o = osb.tile([128, CW], F32, name="o_sb")
                        nc.vector.tensor_add(o[:], ps_t[(tb, ct)][:],
                                             x1_t[tb][:, c0:c0 + CW])
                        nc.sync.dma_start(
                            out_io[tb * 128:(tb + 1) * 128, c0:c0 + CW], o[:])

        if SA < 8:  # truncated probe build: emit dummy output
            for tb in range(TB):
                nc.any.memset(x1_t[tb][:, 0:1], 0.0)
                nc.sync.dma_start(out_io[tb * 128:(tb + 1) * 128, :],
                                  x1_t[tb][:])

    nc.compile()
    return nc


# --------------------------- host-side prep ---------------------------

def host_prep(cfg: Cfg, x, w_qkv, w_out, w1, w2, w3, g1, g2):
    """Build the per-core input maps (numpy, bf16 weights, mask data)."""
    n, D, H, DFF = cfg.n_cores, cfg.D, cfg.H, cfg.DFF
    S_blk, DC, NK, QB, FFB = cfg.S_blk, cfg.DC, cfg.NK, cfg.QB, cfg.FFB
    GW, CW, CT, W2G, T = cfg.GW, cfg.CW, cfg.CT, cfg.W2G, cfg.T

    def group_layout(w, gw):
        # [D, C] -> [C//gw, 128, DC, gw]
        C = w.shape[1]
        return np.ascontiguousarray(
            w.reshape(DC, 128, C // gw, gw).transpose(2, 1, 0, 3))

    x = np.asarray(x, np.float32)
    g1 = np.asarray(g1, np.float32)
    g2 = np.asarray(g2, np.float32)

    # softmax scale Hd^-0.5 is applied in the kernel's exp (not folded here)
    wqkv = np.asarray(w_qkv, np.float32) * g1[:, None]
    if cfg.fp8_qkv:
        wqkv_t = np.clip(group_layout(wqkv, GW) * S_W,
                         -240.0, 240.0).astype(E4M3)
    else:
        wqkv_t = group_layout(wqkv, GW).astype(BF16)

    wout = np.asarray(w_out, np.float32)
    if cfg.fp8_out:
        wout_t = np.clip(group_layout(wout, CW) * S_W,
                         -240.0, 240.0).astype(E4M3)
    else:
        wout_t = group_layout(wout, CW).astype(BF16)

    w1g = (np.asarray(w1, np.float32) * g2[:, None]).reshape(DC, 128, FFB, 128)
    w3g = (np.asarray(w3, np.float32) * g2[:, None]).reshape(DC, 128, FFB, 128)
    w13 = np.stack([w1g, w3g], axis=3).reshape(DC, 128, 2 * DFF)
    w13_t = group_layout(w13.reshape(DC * 128, 2 * DFF), 512).astype(BF16)

    w2_t = np.ascontiguousarray(
        np.asarray(w2, np.float32).reshape(FFB // W2G, W2G, 128, D)
        .transpose(0, 2, 1, 3)).astype(BF16)

    ident = np.eye(128, dtype=np.float32).astype(BF16)
    ones128 = np.ones((128, 128), np.float32).astype(BF16)

    # local order: l = blk*256 + batch*128 + i ; token = n*(blk*128+i) + c
    l = np.arange(cfg.T)
    blk, qb_a, i_a = l // 256, (l // 128) % 2, l % 128
    base_pos = n * (blk * 128 + i_a)

    in_maps = []
    for c in range(n):
        pos = base_pos + c
        x_c = np.ascontiguousarray(x[qb_a, pos, :])
        # diagonal-chunk masks: key (p, rank r) vs query (q, core c):
        # allowed iff p < q or (p == q and r <= c)
        masks = np.zeros((128, n // 2, 2, 256), np.float32)
        kp = np.arange(128)[:, None]
        qq = np.arange(128)[None, :]
        for r in range(n):
            m = (kp < qq) | ((kp == qq) & (r <= c))
            masks[:, r // 2, r % 2, 0:128] = m
            masks[:, r // 2, r % 2, 128:256] = m
        in_maps.append({
            "x_c": x_c,
            "wqkv_t": wqkv_t, "wout_t": wout_t, "w13_t": w13_t, "w2_t": w2_t,
            "masks": masks.astype(E4M3 if cfg.fp8_att else BF16),
            "ident": ident, "ones128": ones128,
        })
    return in_maps


def assemble(cfg: Cfg, results):
    n = cfg.n_cores
    out = np.empty((cfg.B, cfg.S, cfg.D), np.float32)
    l = np.arange(cfg.T)
    blk, qb_a, i_a = l // 256, (l // 128) % 2, l % 128
    base_pos = n * (blk * 128 + i_a)
    for c in range(n):
        out[qb_a, base_pos + c] = results[c]["out_c"]
    return out


_NC_CACHE = {}


def get_nc(cfg: Cfg = FULL):
    if cfg not in _NC_CACHE:
        _NC_CACHE[cfg] = build_nc(cfg)
    return _NC_CACHE[cfg]


def kernel(x, w_qkv, w_out, w1, w2, w3, g1, g2):
    cfg = FULL
    nc = get_nc(cfg)
    in_maps = host_prep(cfg, x, w_qkv, w_out, w1, w2, w3, g1, g2)
    res = run_bass_kernel_spmd(nc, in_maps, core_ids=list(range(cfg.n_cores)))
    return assemble(cfg, res.results)

